# revision 21
# baseline (speedup 1.0000x reference)
"""LIAFResBlock forward on 8 Trainium2 NeuronCores (data-parallel over batch).

Self-contained: hardcodes shapes for x [16,64,8,56,56] -> out [16,128,8,28,28].

Math notes (vs the PyTorch/JAX reference):
  - conv biases are no-ops: every conv is followed by training-mode BN, which
    subtracts the per-channel mean, absorbing any per-channel constant.
  - the final mem_update on a binary {0,1} tensor is the identity because
    d = sigmoid(0.5) ~ 0.6225 and d*0.5 < 0.5, so out = lif_act(bn2(cv2)+bn_sc(sc)).
  - the first mem_update runs in "normalized" space: with a1 = g1*rstd1 (>0),
    v = m/a1 satisfies v[t] = d*v[t-1]*[v<=tau] + (cv1[t] + beta1/a1),
    spike[t] = v[t] > tau, tau = 0.5/a1. BN1 folds into a per-channel bias on
    cv1 plus a per-channel threshold.
  - the final compare is sign-safe: out = 1[a2*cv2 > (0.5 - bsc - asc*sc) - b2]
    needs no assumption on a2's sign.
  - BN batch stats are global over B=16: each core computes per-channel
    (sum, sumsq) partials; an AllGather + local sum combines them (cheaper
    than AllReduce on trn2 for tiny payloads).

Layout notes:
  - all weights and BN params are packed on the host into one [128, 2822]
    fp32 array ("wpk", declared float32r) so a single contiguous DMA stages
    them (a DMA-transpose of [O,I,K] weights costs ~100us in 4B descriptors).
  - x is pre-padded and parity-split on the host: per (sample, t) the SBUF
    tile is [128, 29, 58] with partitions 0-63 = even input rows (xe[r] =
    x[2r]) and partitions 64-127 = odd rows shifted (xo[r] = x[2r-1], row 0
    zero). For the stride-2 3x3 conv, out row i needs x rows 2i-1 (xo[i]),
    2i (xe[i]), 2i+1 (xo[i+1]), so taps (kh=0,kh=1) pair into one K=128
    matmul at identical free offsets, and kh=2 rides K=128 matmuls with the
    xe half's weights zeroed. Columns are zero-padded to 58 on the host, so
    no on-chip memsets or duplicated HBM reads are needed.
  - all conv matmuls run as float32r (1 cycle/row for >=256 output rows vs 4
    for fp32). f32r rounds each operand to ~12 mantissa bits, which the LIF
    recurrence amplifies ~27x into spike flips, so conv1 is error-compensated
    in 3 f32r passes: w_hi@x_hi + w_hi@x_res + w_res@x_hi, where x_hi is an
    on-chip DVE f32r copy (bit-identical to the PE's rounding, so x_hi+x_res
    is exact) and w_hi is host-rounded to 11 mantissa bits. The shortcut and
    conv2 stay single-pass f32r: their errors hit only the final threshold
    (no recurrence amplification) and cost ~550 of the ~1858 allowed flips.
"""
import math
import sys

import numpy as np

sys.path.insert(0, "/opt/trn_rl_repo")

import concourse.bass as bass  # noqa: E402
import concourse.bacc as bacc  # noqa: E402
import concourse.tile as tile  # noqa: E402
from concourse import mybir  # noqa: E402
from concourse.bass_utils import run_bass_kernel_spmd  # noqa: E402

dt = mybir.dt
Alu = mybir.AluOpType
Act = mybir.ActivationFunctionType

B, CIN, COUT, T, H, W = 16, 64, 128, 8, 56, 56
HO = WO = 28
NPIX = HO * WO          # 784
CHUNK = NPIX // 2       # 392 (one PSUM bank)
NCORES = 8
BPC = B // NCORES       # 2 samples per core
NT = BPC * T            # 16 (s,t) tiles per core
NLOC = BPC * T * NPIX   # 12544 elements/channel per core
NGLOB = B * T * NPIX    # 100352 elements/channel globally
EPS = 1e-5
XR, XC = 29, 58         # parity-split padded x tile rows/cols
HS = WS = HO + 2        # 30x30 padded spike tile

# packed weight/param column offsets (all in fp32 elements).
# conv1 weights are split on the host into an 11-mantissa-bit "hi" part
# (read exactly by the ~12-bit f32r PE datapath) and the fp32 residual, for
# the 3-pass error-compensated conv1: w_hi*x_hi + w_hi*x_res + w_res*x_hi.
O_W1PH = 0              # [128, 3*128]  conv1 hi taps kh=1 (xe half) / kh=0 (xo)
O_W1QH = 384            # [128, 3*128]  conv1 hi taps kh=2 (xo half; xe half 0)
O_W1PR = 768            # [128, 3*128]  conv1 residual taps (pair 1)
O_W1QR = 1152           # [128, 3*128]  conv1 residual taps (pair 2)
O_W2 = 1536             # [128, 9*128]  conv2 lhsT per tap
O_WS = 2688             # [64, 128]     shortcut lhsT (partitions 0-63)
O_PAR = 2816            # [128, 6]      bn1_g, bn1_b, bn2_g, bn2_b, scn_g, scn_b
WPK = O_PAR + 6


def _ap(base, off, free):
    """Sub-view of a 2D/3D SBUF AP: keep partition dim, custom free dims."""
    return bass.AP(tensor=base.tensor, offset=base.offset + off,
                   ap=[base.ap[0]] + free)


def build_nc(d: float) -> bass.Bass:
    nc = bacc.Bacc("TRN2", target_bir_lowering=False, num_devices=NCORES)
    f32r = dt.float32r

    x_d = nc.dram_tensor("x", [BPC, T, 2 * CIN, XR, XC], dt.float32,
                         kind="ExternalInput")
    wpk_d = nc.dram_tensor("wpk", [2 * CIN, WPK], dt.float32r,
                           kind="ExternalInput")
    out_d = nc.dram_tensor("out", [BPC, COUT, T, HO, WO], dt.float32,
                           kind="ExternalOutput")

    from contextlib import ExitStack
    with tile.TileContext(nc) as tc, ExitStack() as stk:
        big = stk.enter_context(tc.tile_pool(name="big", bufs=1))
        const = stk.enter_context(tc.tile_pool(name="const", bufs=1))
        psum = stk.enter_context(tc.tile_pool(name="psum", bufs=8, space="PSUM"))
        dramp = stk.enter_context(tc.tile_pool(name="dramp", bufs=1, space="DRAM"))

        # ---- first x tile DMA ahead of the weight pack (head latency) ----
        xpool_cm = tc.tile_pool(name="xtp", bufs=2)
        xpool = xpool_cm.__enter__()
        xhp_cm = tc.tile_pool(name="xhp", bufs=2)
        xhp = xhp_cm.__enter__()
        xrp_cm = tc.tile_pool(name="xrp", bufs=2)
        xrp = xrp_cm.__enter__()
        xt00 = xpool.tile([2 * CIN, XR, XC], dt.float32, tag="xt")
        nc.sync.dma_start(out=xt00[:, :, :], in_=x_d[0, 0, :, :, :])

        # ---- weights + params: one contiguous DMA ----
        wpk = const.tile([2 * CIN, WPK], f32r)
        nc.sync.dma_start(out=wpk[:, :], in_=wpk_d[:, :])
        wb = wpk[:, 0]
        wbs = wpk[0:CIN, 0]

        def wcol(base, kw):
            return _ap(wb, base + kw * COUT, [[1, COUT]])

        def w2_t(k):
            return _ap(wb, O_W2 + k * COUT, [[1, COUT]])

        ws_t = _ap(wbs, O_WS, [[1, COUT]])
        params = {}
        for i, p in enumerate(["bn1_g", "bn1_b", "bn2_g", "bn2_b",
                               "scn_g", "scn_b"]):
            params[p] = wpk[:, O_PAR + i:O_PAR + i + 1].bitcast(dt.float32)
        eps_t = const.tile([COUT, 1], dt.float32)
        nc.vector.memset(eps_t[:, :], EPS)

        # ---- persistent activation buffers (per-channel partition layout) ----
        cv1f = big.tile([COUT, NLOC], dt.float32)   # conv1 raw, later c' = cv1+beta~
        scf = big.tile([COUT, NLOC], dt.float32)    # shortcut raw, later sc''
        cv2f = cv1f  # conv2 output overwrites c' slices (dead after the v read)
        st1 = const.tile([COUT, 2 * NT, 6], dt.float32)   # bn_stats conv1
        sts = const.tile([COUT, 2 * NT, 6], dt.float32)   # bn_stats shortcut
        st2 = const.tile([COUT, 2 * NT, 6], dt.float32)   # bn_stats conv2

        # persistent spike tiles (rings zeroed once, reused round-robin)
        sps = [big.tile([COUT, HS, WS], f32r, name=f"sp{i}") for i in range(4)]
        for sp in sps:
            for r in (sp[:, 0, :], sp[:, HS - 1, :],
                      _ap(sp[:, 0, 0], 0, [[WS, HS], [WS - 1, 2]])):
                nc.gpsimd.memset(r.bitcast(dt.float32), 0.0)
                nc.gpsimd.tensor_copy(r, r.bitcast(dt.float32))

        # ================= phase A: conv1 + shortcut =================
        if True:
            for s in range(BPC):
                for t in range(T):
                    it = s * T + t
                    if it == 0:
                        xt = xt00
                    else:
                        xt = xpool.tile([2 * CIN, XR, XC], dt.float32, tag="xt")
                        nc.sync.dma_start(out=xt[:, :, :], in_=x_d[s, t, :, :, :])
                    # split x into the f32r-exact hi part (DVE f32r copy ==
                    # PE rounding) and the fp32 residual (Pool), then 3-pass
                    # conv1: w_hi*x_hi + w_hi*x_res + w_res*x_hi
                    xh = xhp.tile([2 * CIN, XR, XC], f32r, tag="xh")
                    nc.vector.tensor_copy(xh[:, :, :], xt[:, :, :])
                    xr = xrp.tile([2 * CIN, XR, XC], f32r, tag="xr")
                    nc.gpsimd.tensor_tensor(xr[:, :, :], xt[:, :, :],
                                            xh[:, :, :].bitcast(dt.float32),
                                            Alu.subtract)
                    xhb = xh[:, 0, 0]        # full 128-partition base
                    xrb = xr[:, 0, 0]
                    xe = xh[0:CIN, 0, 0]     # even-rows half (partitions 0-63)
                    for c in range(2):
                        co = c * 14 * XC
                        ps1 = psum.tile([COUT, CHUNK], dt.float32, tag="mm")
                        passes = [(O_W1PH, O_W1QH, xhb), (O_W1PH, O_W1QH, xrb),
                                  (O_W1PR, O_W1QR, xhb)]
                        n = 0
                        for op_, oq, xb in passes:
                            for kw in range(3):
                                rhs = _ap(xb, kw + co, [[XC, 14], [2, WO]])
                                nc.tensor.matmul(ps1[:, :], wcol(op_, kw), rhs,
                                                 start=(n == 0), stop=False)
                                n += 1
                            for kw in range(3):
                                rhs = _ap(xb, XC + kw + co, [[XC, 14], [2, WO]])
                                nc.tensor.matmul(ps1[:, :], wcol(oq, kw), rhs,
                                                 start=False, stop=(n == 17))
                                n += 1
                        off = it * NPIX + c * CHUNK
                        nc.scalar.copy(cv1f[:, off:off + CHUNK], ps1[:, :])
                        nc.vector.bn_stats(out=st1[:, 2 * it + c, :], in_=ps1[:, :])
                        # shortcut 1x1 stride2: x[2i,2j] = xe row i, padded col 2j+1
                        ps2 = psum.tile([COUT, CHUNK], dt.float32, tag="mm")
                        rhs = _ap(xe, 1 + co, [[XC, 14], [2, WO]])
                        nc.tensor.matmul(ps2[:, :], ws_t, rhs,
                                         start=True, stop=True)
                        nc.scalar.copy(scf[:, off:off + CHUNK], ps2[:, :])
                        nc.vector.bn_stats(out=sts[:, 2 * it + c, :], in_=ps2[:, :])

        xrp_cm.__exit__(None, None, None)
        xhp_cm.__exit__(None, None, None)
        xpool_cm.__exit__(None, None, None)

        # ---- local stats -> (sum, sumsq) -> AllGather #1 + local sum ----
        # ar1 columns: [sum1, sumsq1, sums, sumsqs]
        mv1 = const.tile([COUT, 2, 2], dt.float32)  # [:, {bn1,scn}, {mean,var}]
        nc.vector.bn_aggr(out=mv1[:, 0, :], in_=st1[:, :, :])
        nc.vector.bn_aggr(out=mv1[:, 1, :], in_=sts[:, :, :])
        ar1 = const.tile([COUT, 2, 2], dt.float32)  # [:, {bn1,scn}, {sum,sumsq}]
        mvb = mv1[:, 0, 0]
        arb = ar1[:, 0, 0]
        mean2 = _ap(mvb, 0, [[2, 2]])
        var2 = _ap(mvb, 1, [[2, 2]])
        sum2 = _ap(arb, 0, [[2, 2]])
        ssq2 = _ap(arb, 1, [[2, 2]])
        nc.vector.tensor_scalar_mul(sum2, mean2, float(NLOC))
        nc.vector.scalar_tensor_tensor(ssq2, mean2, float(NLOC), mean2,
                                       Alu.mult, Alu.mult)
        nc.vector.scalar_tensor_tensor(ssq2, var2, float(NLOC), ssq2,
                                       Alu.mult, Alu.add)

        def gather_sum(ar, width, tag):
            """AllGather [COUT,width] partials from 8 cores, sum locally."""
            cci = dramp.tile([COUT, width], dt.float32, tag=tag + "_i")
            cco = dramp.tile([NCORES, COUT, width], dt.float32,
                             addr_space="Shared", tag=tag + "_o")
            nc.sync.dma_start(out=cci[:, :], in_=ar)
            nc.gpsimd.collective_compute(
                "AllGather", Alu.bypass, replica_groups=[list(range(NCORES))],
                ins=[cci[:, :].opt()], outs=[cco[:, :, :].opt()])
            g = const.tile([COUT, NCORES, width], dt.float32, tag=tag + "_g")
            nc.sync.dma_start(out=g[:, :, :],
                              in_=cco[:, :, :].rearrange("r c k -> c r k"))
            gf = g[:, 0, 0]
            h = NCORES * width
            acc = const.tile([COUT, h // 2], dt.float32, tag=tag + "_a")
            nc.vector.tensor_tensor(
                acc[:, :], _ap(gf, 0, [[1, h // 2]]),
                _ap(gf, h // 2, [[1, h // 2]]), Alu.add)
            while h > 2 * width:
                h //= 2
                af = acc[:, 0]
                nc.vector.tensor_tensor(
                    acc[:, 0:h // 2], _ap(af, 0, [[1, h // 2]]),
                    _ap(af, h // 2, [[1, h // 2]]), Alu.add)
            return acc, g  # acc[:, 0:width] holds the global sums

        gs1, g1raw = gather_sum(ar1[:, :, :], 4, "cc1")

        def mk_bn_consts(sums, g, b, tag, ncol=1):
            """global interleaved (sum, sumsq) -> a = g*rstd, bb = b - a*mean.

            sums is an AP pair (sum_cols, sumsq_cols) of ncol columns each;
            g, b are [COUT, ncol] APs. Returns (a, bb) [COUT, ncol] tiles.
            """
            sum_c, ssq_c = sums
            mean = const.tile([COUT, ncol], dt.float32, tag=tag + "_mean")
            nc.vector.tensor_scalar_mul(mean[:, :], sum_c, 1.0 / NGLOB)
            var = const.tile([COUT, ncol], dt.float32, tag=tag + "_var")
            nc.vector.tensor_scalar_mul(var[:, :], ssq_c, 1.0 / NGLOB)
            m2 = const.tile([COUT, ncol], dt.float32, tag=tag + "_m2")
            nc.vector.tensor_tensor(m2[:, :], mean[:, :], mean[:, :], Alu.mult)
            nc.vector.tensor_tensor(var[:, :], var[:, :], m2[:, :], Alu.subtract)
            a = const.tile([COUT, ncol], dt.float32, tag=tag + "_a")
            nc.scalar.activation(a[:, :], var[:, :], Act.Sqrt,
                                 bias=eps_t[:, 0:1])
            nc.vector.reciprocal(a[:, :], a[:, :])
            nc.vector.tensor_tensor(a[:, :], a[:, :], g, Alu.mult)
            bb = const.tile([COUT, ncol], dt.float32, tag=tag + "_bb")
            nc.vector.tensor_tensor(bb[:, :], a[:, :], mean[:, :], Alu.mult)
            nc.vector.tensor_tensor(bb[:, :], b, bb[:, :], Alu.subtract)
            return a, bb

        # bn1 + scn in one 2-column pass: gs1 cols are [s1, q1, ss, qs];
        # params (bn1_g, scn_g) and (bn1_b, scn_b) are stride-4 pairs in wpk
        g1b = gs1[:, 0]
        a12, b12 = mk_bn_consts(
            (_ap(g1b, 0, [[2, 2]]), _ap(g1b, 1, [[2, 2]])),
            _ap(wb, O_PAR + 0, [[4, 2]]).bitcast(dt.float32),
            _ap(wb, O_PAR + 1, [[4, 2]]).bitcast(dt.float32),
            "bn1scn", ncol=2)
        a1, asc = a12[:, 0:1], a12[:, 1:2]
        b1, bsc = b12[:, 0:1], b12[:, 1:2]

        # tau = 0.5/a1 ; beta~ = b1/a1  (a1 > 0 since gamma=1 at init)
        ra1 = const.tile([COUT, 1], dt.float32)
        nc.vector.reciprocal(ra1[:, :], a1)
        tau = const.tile([COUT, 1], dt.float32)
        nc.vector.tensor_scalar_mul(tau[:, :], ra1[:, :], 0.5)
        btil = const.tile([COUT, 1], dt.float32)
        nc.vector.tensor_tensor(btil[:, :], b1, ra1[:, :], Alu.mult)
        # for the phase-B shortcut fold: sc'' = -asc*sc + (0.5 - bsc)
        nasc = const.tile([COUT, 1], dt.float32)
        nc.vector.tensor_scalar_mul(nasc[:, :], asc, -1.0)
        c1 = const.tile([COUT, 1], dt.float32)
        nc.vector.tensor_scalar(c1[:, :], bsc, -1.0, 0.5, Alu.mult, Alu.add)

        # keep the PE pstate warm through the AR1 window: a few fp32 dummy
        # matmuls gated on the gathered stats (g tile) run right after the
        # collective lands, bridging the gap until the first conv2 is ready.
        for i in range(2):
            pw = psum.tile([COUT, CHUNK], dt.float32, tag="mm")
            nc.tensor.matmul(pw[:, :], wpk[:, 0:COUT].bitcast(dt.float32),
                             _ap(g1raw[:, 0, 0], 0, [[0, 14], [1, 28]]),
                             start=True, stop=True)

        # ================= phase B: LIF recurrence + conv2 =================
        # Software-pipelined with 2-tile lookahead: the recurrence + spike for
        # tile i+2 is emitted before tile i's PSUM copies, so on the in-order
        # Pool/DVE queues spikes never wait behind copies that depend on the
        # PE, and the PE never starves.
        tiles = [(t, s) for t in range(T) for s in range(BPC)]
        v_prev = [None] * BPC
        spof = [None] * len(tiles)

        with tc.tile_pool(name="phu", bufs=3) as pu, \
             tc.tile_pool(name="phv", bufs=5) as pv:

            def emit_front(i):
                """fold + recurrence + spike for tile i."""
                t, s = tiles[i]
                off = (s * T + t) * NPIX
                cslice = cv1f[:, off:off + NPIX]
                nc.scalar.activation(cslice, cslice, Act.Identity,
                                     bias=btil[:, :])
                if t == 0:
                    v = cslice
                else:
                    u = pu.tile([COUT, NPIX], dt.float32, tag="u")
                    nc.vector.scalar_tensor_tensor(
                        u[:, :], v_prev[s], tau[:, :], v_prev[s],
                        Alu.is_le, Alu.mult)
                    vt = pv.tile([COUT, NPIX], dt.float32, tag="v")
                    nc.vector.scalar_tensor_tensor(
                        vt[:, :], u[:, :], float(d), cslice, Alu.mult, Alu.add)
                    v = vt[:, :]
                v_prev[s] = v
                sp = sps[i % 4]
                spof[i] = sp
                spi = _ap(sp[:, 0, 0], WS + 1, [[WS, HO], [1, WO]])
                nc.gpsimd.tensor_scalar(spi, v, tau[:, :], None, Alu.is_gt)

            def emit_back(i):
                """conv2 matmuls + copies + stats + shortcut fold for tile i."""
                t, s = tiles[i]
                it = s * T + t
                off = it * NPIX
                spb = spof[i][:, 0, 0]
                for c in range(2):
                    ps3 = psum.tile([COUT, CHUNK], dt.float32, tag="mm")
                    for k in range(9):
                        kh, kw = divmod(k, 3)
                        rhs = _ap(spb, kh * WS + kw + c * 14 * WS,
                                  [[WS, 14], [1, WO]])
                        nc.tensor.matmul(ps3[:, :], w2_t(k), rhs,
                                         start=(k == 0), stop=(k == 8))
                    o2 = off + c * CHUNK
                    nc.scalar.copy(cv2f[:, o2:o2 + CHUNK], ps3[:, :])
                    nc.vector.bn_stats(out=st2[:, 2 * it + c, :], in_=ps3[:, :])
                # sc'' = -asc*sc + (0.5 - bsc)  (Act slack during phase B)
                nc.scalar.activation(scf[:, off:off + NPIX],
                                     scf[:, off:off + NPIX], Act.Identity,
                                     scale=nasc[:, :], bias=c1[:, :])

            emit_front(0)
            emit_front(1)
            for i in range(len(tiles)):
                if i + 2 < len(tiles):
                    emit_front(i + 2)
                emit_back(i)

        # ---- AllGather #2 (bn2 stats) ----
        mv2 = const.tile([COUT, 2], dt.float32)
        nc.vector.bn_aggr(out=mv2[:, :], in_=st2[:, :, :])
        ar2 = const.tile([COUT, 2], dt.float32)
        nc.vector.tensor_scalar_mul(ar2[:, 0:1], mv2[:, 0:1], float(NLOC))
        nc.vector.scalar_tensor_tensor(ar2[:, 1:2], mv2[:, 0:1], float(NLOC),
                                       mv2[:, 0:1], Alu.mult, Alu.mult)
        nc.vector.scalar_tensor_tensor(ar2[:, 1:2], mv2[:, 1:2], float(NLOC),
                                       ar2[:, 1:2], Alu.mult, Alu.add)
        gs2, _ = gather_sum(ar2[:, :], 2, "cc2")

        a2t, b2t = mk_bn_consts((gs2[:, 0:1], gs2[:, 1:2]),
                                params["bn2_g"], params["bn2_b"], "bn2")
        a2 = a2t[:, 0:1]
        negb2 = const.tile([COUT, 1], dt.float32)
        nc.vector.tensor_scalar_mul(negb2[:, :], b2t[:, 0:1], -1.0)

        # out = 1[a2*cv2 > sc'' - b2]
        with tc.tile_pool(name="outp", bufs=5) as op, \
             tc.tile_pool(name="thp", bufs=5) as tp:
            for s in range(BPC):
                for t in range(T):
                    off = (s * T + t) * NPIX
                    thr = tp.tile([COUT, NPIX], dt.float32, tag="th")
                    if t % 2 == 0:
                        nc.scalar.activation(thr[:, :], scf[:, off:off + NPIX],
                                             Act.Identity, bias=negb2[:, :])
                    else:
                        nc.gpsimd.tensor_scalar(thr[:, :],
                                                scf[:, off:off + NPIX],
                                                negb2[:, :], None, Alu.add)
                    ot = op.tile([COUT, NPIX], dt.float32, tag="ot")
                    nc.vector.scalar_tensor_tensor(
                        ot[:, :], cv2f[:, off:off + NPIX], a2,
                        thr[:, :], Alu.mult, Alu.is_gt)
                    nc.sync.dma_start(
                        out=out_d.ap()[s, :, t, :, :].rearrange("c h w -> c (h w)"),
                        in_=ot[:, :])

    nc.compile()
    return nc


def _prep_inputs(inputs):
    """Host-side restaging: parity-split padded x + packed transposed weights."""
    x = np.ascontiguousarray(inputs["x"], dtype=np.float32)
    xt = x.transpose(0, 2, 1, 3, 4)  # [B, T, C, H, W]
    xeo = np.zeros((B, T, 2 * CIN, XR, XC), dtype=np.float32)
    xeo[:, :, 0:CIN, 0:28, 1:57] = xt[:, :, :, 0::2, :]
    xeo[:, :, CIN:2 * CIN, 1:29, 1:57] = xt[:, :, :, 1::2, :]

    w1 = np.ascontiguousarray(inputs["cv1_w"], np.float32).reshape(COUT, CIN, 9)
    w2 = np.ascontiguousarray(inputs["cv2_w"], np.float32).reshape(COUT, COUT, 9)
    ws = np.ascontiguousarray(inputs["sc_w"], np.float32).reshape(COUT, CIN)
    wpk = np.zeros((2 * CIN, WPK), np.float32)
    # parity-paired conv1 lhsT: kh=1 taps on xe half (partitions 0-63),
    # kh=0 on xo half; kh=2 on xo half with the xe half zero
    w1p = np.zeros((2 * CIN, 384), np.float32)
    w1p[0:CIN] = w1[:, :, 3:6].transpose(1, 2, 0).reshape(CIN, 384)
    w1p[CIN:] = w1[:, :, 0:3].transpose(1, 2, 0).reshape(CIN, 384)
    w1q = np.zeros((2 * CIN, 384), np.float32)
    w1q[CIN:] = w1[:, :, 6:9].transpose(1, 2, 0).reshape(CIN, 384)

    def rnd11(a):
        u = a.view(np.uint32)
        return ((u + np.uint32(1 << 11)) & np.uint32(0xFFFFF000)).view(np.float32)

    for full, ohi, ores in ((w1p, O_W1PH, O_W1PR), (w1q, O_W1QH, O_W1QR)):
        hi = rnd11(full)
        wpk[:, ohi:ohi + 384] = hi
        wpk[:, ores:ores + 384] = full - hi
    wpk[:, O_W2:O_W2 + 1152] = w2.transpose(1, 2, 0).reshape(COUT, 1152)
    wpk[0:CIN, O_WS:O_WS + COUT] = ws.T
    for i, p in enumerate(["bn1_g", "bn1_b", "bn2_g", "bn2_b",
                           "scn_g", "scn_b"]):
        wpk[:, O_PAR + i] = np.asarray(inputs[p], np.float32).ravel()
    return xeo, wpk


_CACHE = {}


def kernel(**inputs):
    xeo, wpk = _prep_inputs(inputs)
    d = float(1.0 / (1.0 + math.exp(-float(np.asarray(inputs["decay"]).ravel()[0]))))

    key = round(d, 12)
    if key not in _CACHE:
        _CACHE[key] = build_nc(d)
    nc = _CACHE[key]

    in_maps = [{"x": xeo[c * BPC:(c + 1) * BPC], "wpk": wpk}
               for c in range(NCORES)]
    res = run_bass_kernel_spmd(nc, in_maps, core_ids=list(range(NCORES)))
    out = np.concatenate([res.results[c]["out"] for c in range(NCORES)], axis=0)
    return out.astype(np.float32)


# revision 23
# speedup vs baseline: 1.0079x; 1.0079x over previous
"""LIAFResBlock forward on 8 Trainium2 NeuronCores (data-parallel over batch).

Self-contained: hardcodes shapes for x [16,64,8,56,56] -> out [16,128,8,28,28].

Math notes (vs the PyTorch/JAX reference):
  - conv biases are no-ops: every conv is followed by training-mode BN, which
    subtracts the per-channel mean, absorbing any per-channel constant.
  - the final mem_update on a binary {0,1} tensor is the identity because
    d = sigmoid(0.5) ~ 0.6225 and d*0.5 < 0.5, so out = lif_act(bn2(cv2)+bn_sc(sc)).
  - the first mem_update runs in "normalized" space: with a1 = g1*rstd1 (>0),
    v = m/a1 satisfies v[t] = d*v[t-1]*[v<=tau] + (cv1[t] + beta1/a1),
    spike[t] = v[t] > tau, tau = 0.5/a1. BN1 folds into a per-channel bias on
    cv1 plus a per-channel threshold.
  - the final compare is sign-safe: out = 1[a2*cv2 > (0.5 - bsc - asc*sc) - b2]
    needs no assumption on a2's sign.
  - BN batch stats are global over B=16: each core computes per-channel
    (sum, sumsq) partials; an AllGather + local sum combines them (cheaper
    than AllReduce on trn2 for tiny payloads).

Layout notes:
  - all weights and BN params are packed on the host into one [128, 2822]
    fp32 array ("wpk", declared float32r) so a single contiguous DMA stages
    them (a DMA-transpose of [O,I,K] weights costs ~100us in 4B descriptors).
  - x is pre-padded and parity-split on the host: per (sample, t) the SBUF
    tile is [128, 29, 58] with partitions 0-63 = even input rows (xe[r] =
    x[2r]) and partitions 64-127 = odd rows shifted (xo[r] = x[2r-1], row 0
    zero). For the stride-2 3x3 conv, out row i needs x rows 2i-1 (xo[i]),
    2i (xe[i]), 2i+1 (xo[i+1]), so taps (kh=0,kh=1) pair into one K=128
    matmul at identical free offsets, and kh=2 rides K=128 matmuls with the
    xe half's weights zeroed. Columns are zero-padded to 58 on the host, so
    no on-chip memsets or duplicated HBM reads are needed.
  - all conv matmuls run as float32r (1 cycle/row for >=256 output rows vs 4
    for fp32). f32r rounds each operand to ~12 mantissa bits, which the LIF
    recurrence amplifies ~27x into spike flips, so conv1 is error-compensated
    in 3 f32r passes: w_hi@x_hi + w_hi@x_res + w_res@x_hi, where x_hi is an
    on-chip DVE f32r copy (bit-identical to the PE's rounding, so x_hi+x_res
    is exact) and w_hi is host-rounded to 11 mantissa bits. The shortcut and
    conv2 stay single-pass f32r: their errors hit only the final threshold
    (no recurrence amplification) and cost ~550 of the ~1858 allowed flips.
"""
import math
import sys

import numpy as np

sys.path.insert(0, "/opt/trn_rl_repo")

import concourse.bass as bass  # noqa: E402
import concourse.bacc as bacc  # noqa: E402
import concourse.tile as tile  # noqa: E402
from concourse import mybir  # noqa: E402
from concourse.bass_utils import run_bass_kernel_spmd  # noqa: E402

dt = mybir.dt
Alu = mybir.AluOpType
Act = mybir.ActivationFunctionType

B, CIN, COUT, T, H, W = 16, 64, 128, 8, 56, 56
HO = WO = 28
NPIX = HO * WO          # 784
CHUNK = NPIX // 2       # 392 (one PSUM bank)
NCORES = 8
BPC = B // NCORES       # 2 samples per core
NT = BPC * T            # 16 (s,t) tiles per core
NLOC = BPC * T * NPIX   # 12544 elements/channel per core
NGLOB = B * T * NPIX    # 100352 elements/channel globally
EPS = 1e-5
XR, XC = 29, 58         # parity-split padded x tile rows/cols
HS = WS = HO + 2        # 30x30 padded spike tile

# packed weight/param column offsets (all in fp32 elements).
# conv1 weights are split on the host into an 11-mantissa-bit "hi" part
# (read exactly by the ~12-bit f32r PE datapath) and the fp32 residual, for
# the 3-pass error-compensated conv1: w_hi*x_hi + w_hi*x_res + w_res*x_hi.
O_W1PH = 0              # [128, 3*128]  conv1 hi taps kh=1 (xe half) / kh=0 (xo)
O_W1QH = 384            # [128, 3*128]  conv1 hi taps kh=2 (xo half; xe half 0)
O_W1PR = 768            # [128, 3*128]  conv1 residual taps (pair 1)
O_W1QR = 1152           # [128, 3*128]  conv1 residual taps (pair 2)
O_W2 = 1536             # [128, 9*128]  conv2 lhsT per tap
O_WS = 2688             # [64, 128]     shortcut lhsT (partitions 0-63)
O_PAR = 2816            # [128, 6]      bn1_g, bn1_b, bn2_g, bn2_b, scn_g, scn_b
WPK = O_PAR + 6


def _ap(base, off, free):
    """Sub-view of a 2D/3D SBUF AP: keep partition dim, custom free dims."""
    return bass.AP(tensor=base.tensor, offset=base.offset + off,
                   ap=[base.ap[0]] + free)


def build_nc(d: float) -> bass.Bass:
    nc = bacc.Bacc("TRN2", target_bir_lowering=False, num_devices=NCORES)
    f32r = dt.float32r

    x_d = nc.dram_tensor("x", [BPC, T, 2 * CIN, XR, XC], dt.float32,
                         kind="ExternalInput")
    wpk_d = nc.dram_tensor("wpk", [2 * CIN, WPK], dt.float32r,
                           kind="ExternalInput")
    out_d = nc.dram_tensor("out", [BPC, COUT, T, HO, WO], dt.float32,
                           kind="ExternalOutput")

    from contextlib import ExitStack
    with tile.TileContext(nc) as tc, ExitStack() as stk:
        big = stk.enter_context(tc.tile_pool(name="big", bufs=1))
        const = stk.enter_context(tc.tile_pool(name="const", bufs=1))
        psum = stk.enter_context(tc.tile_pool(name="psum", bufs=8, space="PSUM"))
        dramp = stk.enter_context(tc.tile_pool(name="dramp", bufs=1, space="DRAM"))

        # ---- first x tile DMA ahead of the weight pack (head latency) ----
        xpool_cm = tc.tile_pool(name="xtp", bufs=2)
        xpool = xpool_cm.__enter__()
        xhp_cm = tc.tile_pool(name="xhp", bufs=2)
        xhp = xhp_cm.__enter__()
        xrp_cm = tc.tile_pool(name="xrp", bufs=2)
        xrp = xrp_cm.__enter__()
        xt00 = xpool.tile([2 * CIN, XR, XC], dt.float32, tag="xt")
        nc.sync.dma_start(out=xt00[:, :, :], in_=x_d[0, 0, :, :, :])

        # ---- weights + params: conv1 part first, rest lands during tile 0
        wpk = const.tile([2 * CIN, WPK], f32r)
        nc.sync.dma_start(out=wpk[:, 0:O_W2], in_=wpk_d[:, 0:O_W2])
        nc.sync.dma_start(out=wpk[:, O_W2:WPK], in_=wpk_d[:, O_W2:WPK])
        wb = wpk[:, 0]
        wbs = wpk[0:CIN, 0]

        def wcol(base, kw):
            return _ap(wb, base + kw * COUT, [[1, COUT]])

        def w2_t(k):
            return _ap(wb, O_W2 + k * COUT, [[1, COUT]])

        ws_t = _ap(wbs, O_WS, [[1, COUT]])
        params = {}
        for i, p in enumerate(["bn1_g", "bn1_b", "bn2_g", "bn2_b",
                               "scn_g", "scn_b"]):
            params[p] = wpk[:, O_PAR + i:O_PAR + i + 1].bitcast(dt.float32)
        eps_t = const.tile([COUT, 1], dt.float32)
        nc.vector.memset(eps_t[:, :], EPS)

        # ---- persistent activation buffers (per-channel partition layout) ----
        cv1f = big.tile([COUT, NLOC], dt.float32)   # conv1 raw, later c' = cv1+beta~
        scf = big.tile([COUT, NLOC], dt.float32)    # shortcut raw, later sc''
        cv2f = cv1f  # conv2 output overwrites c' slices (dead after the v read)
        st1 = const.tile([COUT, 2 * NT, 6], dt.float32)   # bn_stats conv1
        sts = const.tile([COUT, 2 * NT, 6], dt.float32)   # bn_stats shortcut
        st2 = const.tile([COUT, 2 * NT, 6], dt.float32)   # bn_stats conv2

        # persistent spike tiles (rings zeroed once, reused round-robin)
        sps = [big.tile([COUT, HS, WS], f32r, name=f"sp{i}") for i in range(4)]
        for sp in sps:
            for r in (sp[:, 0, :], sp[:, HS - 1, :],
                      _ap(sp[:, 0, 0], 0, [[WS, HS], [WS - 1, 2]])):
                nc.gpsimd.memset(r.bitcast(dt.float32), 0.0)
                nc.gpsimd.tensor_copy(r, r.bitcast(dt.float32))

        # ================= phase A: conv1 + shortcut =================
        if True:
            for s in range(BPC):
                for t in range(T):
                    it = s * T + t
                    if it == 0:
                        xt = xt00
                    else:
                        xt = xpool.tile([2 * CIN, XR, XC], dt.float32, tag="xt")
                        nc.sync.dma_start(out=xt[:, :, :], in_=x_d[s, t, :, :, :])
                    # split x into the f32r-exact hi part (DVE f32r copy ==
                    # PE rounding) and the fp32 residual (Pool), then 3-pass
                    # conv1: w_hi*x_hi + w_hi*x_res + w_res*x_hi
                    xh = xhp.tile([2 * CIN, XR, XC], f32r, tag="xh")
                    nc.vector.tensor_copy(xh[:, :, :], xt[:, :, :])
                    xr = xrp.tile([2 * CIN, XR, XC], f32r, tag="xr")
                    nc.gpsimd.tensor_tensor(xr[:, :, :], xt[:, :, :],
                                            xh[:, :, :].bitcast(dt.float32),
                                            Alu.subtract)
                    xhb = xh[:, 0, 0]        # full 128-partition base
                    xrb = xr[:, 0, 0]
                    xe = xh[0:CIN, 0, 0]     # even-rows half (partitions 0-63)
                    for c in range(2):
                        co = c * 14 * XC
                        ps1 = psum.tile([COUT, CHUNK], dt.float32, tag="mm")
                        passes = [(O_W1PH, O_W1QH, xhb), (O_W1PH, O_W1QH, xrb),
                                  (O_W1PR, O_W1QR, xhb)]
                        n = 0
                        for op_, oq, xb in passes:
                            for kw in range(3):
                                rhs = _ap(xb, kw + co, [[XC, 14], [2, WO]])
                                nc.tensor.matmul(ps1[:, :], wcol(op_, kw), rhs,
                                                 start=(n == 0), stop=False)
                                n += 1
                            for kw in range(3):
                                rhs = _ap(xb, XC + kw + co, [[XC, 14], [2, WO]])
                                nc.tensor.matmul(ps1[:, :], wcol(oq, kw), rhs,
                                                 start=False, stop=(n == 17))
                                n += 1
                        off = it * NPIX + c * CHUNK
                        nc.scalar.copy(cv1f[:, off:off + CHUNK], ps1[:, :])
                        nc.vector.bn_stats(out=st1[:, 2 * it + c, :], in_=ps1[:, :])
                        # shortcut 1x1 stride2: x[2i,2j] = xe row i, padded col 2j+1
                        ps2 = psum.tile([COUT, CHUNK], dt.float32, tag="mm")
                        rhs = _ap(xe, 1 + co, [[XC, 14], [2, WO]])
                        nc.tensor.matmul(ps2[:, :], ws_t, rhs,
                                         start=True, stop=True)
                        nc.scalar.copy(scf[:, off:off + CHUNK], ps2[:, :])
                        nc.vector.bn_stats(out=sts[:, 2 * it + c, :], in_=ps2[:, :])

        xrp_cm.__exit__(None, None, None)
        xhp_cm.__exit__(None, None, None)
        xpool_cm.__exit__(None, None, None)

        # ---- local stats -> (sum, sumsq) -> AllGather #1 + local sum ----
        # ar1 columns: [sum1, sumsq1, sums, sumsqs]
        mv1 = const.tile([COUT, 2, 2], dt.float32)  # [:, {bn1,scn}, {mean,var}]
        nc.vector.bn_aggr(out=mv1[:, 0, :], in_=st1[:, :, :])
        nc.vector.bn_aggr(out=mv1[:, 1, :], in_=sts[:, :, :])
        ar1 = const.tile([COUT, 2, 2], dt.float32)  # [:, {bn1,scn}, {sum,sumsq}]
        mvb = mv1[:, 0, 0]
        arb = ar1[:, 0, 0]
        mean2 = _ap(mvb, 0, [[2, 2]])
        var2 = _ap(mvb, 1, [[2, 2]])
        sum2 = _ap(arb, 0, [[2, 2]])
        ssq2 = _ap(arb, 1, [[2, 2]])
        nc.vector.tensor_scalar_mul(sum2, mean2, float(NLOC))
        nc.vector.scalar_tensor_tensor(ssq2, mean2, float(NLOC), mean2,
                                       Alu.mult, Alu.mult)
        nc.vector.scalar_tensor_tensor(ssq2, var2, float(NLOC), ssq2,
                                       Alu.mult, Alu.add)

        def gather_sum(ar, width, tag):
            """AllGather [COUT,width] partials from 8 cores, sum locally."""
            cci = dramp.tile([COUT, width], dt.float32, tag=tag + "_i")
            cco = dramp.tile([NCORES, COUT, width], dt.float32,
                             addr_space="Shared", tag=tag + "_o")
            nc.sync.dma_start(out=cci[:, :], in_=ar)
            nc.gpsimd.collective_compute(
                "AllGather", Alu.bypass, replica_groups=[list(range(NCORES))],
                ins=[cci[:, :].opt()], outs=[cco[:, :, :].opt()])
            g = const.tile([COUT, NCORES, width], dt.float32, tag=tag + "_g")
            nc.sync.dma_start(out=g[:, :, :],
                              in_=cco[:, :, :].rearrange("r c k -> c r k"))
            gf = g[:, 0, 0]
            h = NCORES * width
            acc = const.tile([COUT, h // 2], dt.float32, tag=tag + "_a")
            nc.vector.tensor_tensor(
                acc[:, :], _ap(gf, 0, [[1, h // 2]]),
                _ap(gf, h // 2, [[1, h // 2]]), Alu.add)
            while h > 2 * width:
                h //= 2
                af = acc[:, 0]
                nc.vector.tensor_tensor(
                    acc[:, 0:h // 2], _ap(af, 0, [[1, h // 2]]),
                    _ap(af, h // 2, [[1, h // 2]]), Alu.add)
            return acc, g  # acc[:, 0:width] holds the global sums

        gs1, g1raw = gather_sum(ar1[:, :, :], 4, "cc1")

        def mk_bn_consts(sums, g, b, tag, ncol=1):
            """global interleaved (sum, sumsq) -> a = g*rstd, bb = b - a*mean.

            sums is an AP pair (sum_cols, sumsq_cols) of ncol columns each;
            g, b are [COUT, ncol] APs. Returns (a, bb) [COUT, ncol] tiles.
            """
            sum_c, ssq_c = sums
            mean = const.tile([COUT, ncol], dt.float32, tag=tag + "_mean")
            nc.vector.tensor_scalar_mul(mean[:, :], sum_c, 1.0 / NGLOB)
            var = const.tile([COUT, ncol], dt.float32, tag=tag + "_var")
            nc.vector.tensor_scalar_mul(var[:, :], ssq_c, 1.0 / NGLOB)
            m2 = const.tile([COUT, ncol], dt.float32, tag=tag + "_m2")
            nc.vector.tensor_tensor(m2[:, :], mean[:, :], mean[:, :], Alu.mult)
            nc.vector.tensor_tensor(var[:, :], var[:, :], m2[:, :], Alu.subtract)
            a = const.tile([COUT, ncol], dt.float32, tag=tag + "_a")
            nc.scalar.activation(a[:, :], var[:, :], Act.Sqrt,
                                 bias=eps_t[:, 0:1])
            nc.vector.reciprocal(a[:, :], a[:, :])
            nc.vector.tensor_tensor(a[:, :], a[:, :], g, Alu.mult)
            bb = const.tile([COUT, ncol], dt.float32, tag=tag + "_bb")
            nc.vector.tensor_tensor(bb[:, :], a[:, :], mean[:, :], Alu.mult)
            nc.vector.tensor_tensor(bb[:, :], b, bb[:, :], Alu.subtract)
            return a, bb

        # bn1 + scn in one 2-column pass: gs1 cols are [s1, q1, ss, qs];
        # params (bn1_g, scn_g) and (bn1_b, scn_b) are stride-4 pairs in wpk
        g1b = gs1[:, 0]
        a12, b12 = mk_bn_consts(
            (_ap(g1b, 0, [[2, 2]]), _ap(g1b, 1, [[2, 2]])),
            _ap(wb, O_PAR + 0, [[4, 2]]).bitcast(dt.float32),
            _ap(wb, O_PAR + 1, [[4, 2]]).bitcast(dt.float32),
            "bn1scn", ncol=2)
        a1, asc = a12[:, 0:1], a12[:, 1:2]
        b1, bsc = b12[:, 0:1], b12[:, 1:2]

        # tau = 0.5/a1 ; beta~ = b1/a1  (a1 > 0 since gamma=1 at init)
        ra1 = const.tile([COUT, 1], dt.float32)
        nc.vector.reciprocal(ra1[:, :], a1)
        tau = const.tile([COUT, 1], dt.float32)
        nc.vector.tensor_scalar_mul(tau[:, :], ra1[:, :], 0.5)
        btil = const.tile([COUT, 1], dt.float32)
        nc.vector.tensor_tensor(btil[:, :], b1, ra1[:, :], Alu.mult)
        # for the phase-B shortcut fold: sc'' = -asc*sc + (0.5 - bsc)
        nasc = const.tile([COUT, 1], dt.float32)
        nc.vector.tensor_scalar_mul(nasc[:, :], asc, -1.0)
        c1 = const.tile([COUT, 1], dt.float32)
        nc.vector.tensor_scalar(c1[:, :], bsc, -1.0, 0.5, Alu.mult, Alu.add)

        # keep the PE pstate warm through the AR1 window: a few fp32 dummy
        # matmuls gated on the gathered stats (g tile) run right after the
        # collective lands, bridging the gap until the first conv2 is ready.
        for i in range(2):
            pw = psum.tile([COUT, CHUNK], dt.float32, tag="mm")
            nc.tensor.matmul(pw[:, :], wpk[:, 0:COUT].bitcast(dt.float32),
                             _ap(g1raw[:, 0, 0], 0, [[0, 14], [1, 28]]),
                             start=True, stop=True)

        # ================= phase B: LIF recurrence + conv2 =================
        # Software-pipelined with 2-tile lookahead: the recurrence + spike for
        # tile i+2 is emitted before tile i's PSUM copies, so on the in-order
        # Pool/DVE queues spikes never wait behind copies that depend on the
        # PE, and the PE never starves.
        tiles = [(t, s) for t in range(T) for s in range(BPC)]
        v_prev = [None] * BPC
        spof = [None] * len(tiles)

        with tc.tile_pool(name="phu", bufs=3) as pu, \
             tc.tile_pool(name="phv", bufs=5) as pv:

            def emit_front(i):
                """fold + recurrence + spike for tile i."""
                t, s = tiles[i]
                off = (s * T + t) * NPIX
                cslice = cv1f[:, off:off + NPIX]
                nc.scalar.activation(cslice, cslice, Act.Identity,
                                     bias=btil[:, :])
                if t == 0:
                    v = cslice
                else:
                    u = pu.tile([COUT, NPIX], dt.float32, tag="u")
                    nc.vector.scalar_tensor_tensor(
                        u[:, :], v_prev[s], tau[:, :], v_prev[s],
                        Alu.is_le, Alu.mult)
                    vt = pv.tile([COUT, NPIX], dt.float32, tag="v")
                    nc.vector.scalar_tensor_tensor(
                        vt[:, :], u[:, :], float(d), cslice, Alu.mult, Alu.add)
                    v = vt[:, :]
                v_prev[s] = v
                sp = sps[i % 4]
                spof[i] = sp
                spi = _ap(sp[:, 0, 0], WS + 1, [[WS, HO], [1, WO]])
                nc.gpsimd.tensor_scalar(spi, v, tau[:, :], None, Alu.is_gt)

            def emit_back(i):
                """conv2 matmuls + copies + stats + shortcut fold for tile i."""
                t, s = tiles[i]
                it = s * T + t
                off = it * NPIX
                spb = spof[i][:, 0, 0]
                for c in range(2):
                    ps3 = psum.tile([COUT, CHUNK], dt.float32, tag="mm")
                    for k in range(9):
                        kh, kw = divmod(k, 3)
                        rhs = _ap(spb, kh * WS + kw + c * 14 * WS,
                                  [[WS, 14], [1, WO]])
                        nc.tensor.matmul(ps3[:, :], w2_t(k), rhs,
                                         start=(k == 0), stop=(k == 8))
                    o2 = off + c * CHUNK
                    nc.scalar.copy(cv2f[:, o2:o2 + CHUNK], ps3[:, :])
                    nc.vector.bn_stats(out=st2[:, 2 * it + c, :], in_=ps3[:, :])
                # sc'' = -asc*sc + (0.5 - bsc)  (Act slack during phase B)
                nc.scalar.activation(scf[:, off:off + NPIX],
                                     scf[:, off:off + NPIX], Act.Identity,
                                     scale=nasc[:, :], bias=c1[:, :])

            emit_front(0)
            emit_front(1)
            for i in range(len(tiles)):
                if i + 2 < len(tiles):
                    emit_front(i + 2)
                emit_back(i)

        # ---- AllGather #2 (bn2 stats) ----
        mv2 = const.tile([COUT, 2], dt.float32)
        nc.vector.bn_aggr(out=mv2[:, :], in_=st2[:, :, :])
        ar2 = const.tile([COUT, 2], dt.float32)
        nc.vector.tensor_scalar_mul(ar2[:, 0:1], mv2[:, 0:1], float(NLOC))
        nc.vector.scalar_tensor_tensor(ar2[:, 1:2], mv2[:, 0:1], float(NLOC),
                                       mv2[:, 0:1], Alu.mult, Alu.mult)
        nc.vector.scalar_tensor_tensor(ar2[:, 1:2], mv2[:, 1:2], float(NLOC),
                                       ar2[:, 1:2], Alu.mult, Alu.add)
        gs2, _ = gather_sum(ar2[:, :], 2, "cc2")

        a2t, b2t = mk_bn_consts((gs2[:, 0:1], gs2[:, 1:2]),
                                params["bn2_g"], params["bn2_b"], "bn2")
        a2 = a2t[:, 0:1]
        negb2 = const.tile([COUT, 1], dt.float32)
        nc.vector.tensor_scalar_mul(negb2[:, :], b2t[:, 0:1], -1.0)

        # out = 1[a2*cv2 > sc'' - b2]
        with tc.tile_pool(name="outp", bufs=5) as op, \
             tc.tile_pool(name="thp", bufs=5) as tp:
            for s in range(BPC):
                for t in range(T):
                    off = (s * T + t) * NPIX
                    thr = tp.tile([COUT, NPIX], dt.float32, tag="th")
                    if t % 2 == 0:
                        nc.scalar.activation(thr[:, :], scf[:, off:off + NPIX],
                                             Act.Identity, bias=negb2[:, :])
                    else:
                        nc.gpsimd.tensor_scalar(thr[:, :],
                                                scf[:, off:off + NPIX],
                                                negb2[:, :], None, Alu.add)
                    ot = op.tile([COUT, NPIX], dt.float32, tag="ot")
                    nc.vector.scalar_tensor_tensor(
                        ot[:, :], cv2f[:, off:off + NPIX], a2,
                        thr[:, :], Alu.mult, Alu.is_gt)
                    nc.sync.dma_start(
                        out=out_d.ap()[s, :, t, :, :].rearrange("c h w -> c (h w)"),
                        in_=ot[:, :])

    nc.compile()
    return nc


def _prep_inputs(inputs):
    """Host-side restaging: parity-split padded x + packed transposed weights."""
    x = np.ascontiguousarray(inputs["x"], dtype=np.float32)
    xt = x.transpose(0, 2, 1, 3, 4)  # [B, T, C, H, W]
    xeo = np.zeros((B, T, 2 * CIN, XR, XC), dtype=np.float32)
    xeo[:, :, 0:CIN, 0:28, 1:57] = xt[:, :, :, 0::2, :]
    xeo[:, :, CIN:2 * CIN, 1:29, 1:57] = xt[:, :, :, 1::2, :]

    w1 = np.ascontiguousarray(inputs["cv1_w"], np.float32).reshape(COUT, CIN, 9)
    w2 = np.ascontiguousarray(inputs["cv2_w"], np.float32).reshape(COUT, COUT, 9)
    ws = np.ascontiguousarray(inputs["sc_w"], np.float32).reshape(COUT, CIN)
    wpk = np.zeros((2 * CIN, WPK), np.float32)
    # parity-paired conv1 lhsT: kh=1 taps on xe half (partitions 0-63),
    # kh=0 on xo half; kh=2 on xo half with the xe half zero
    w1p = np.zeros((2 * CIN, 384), np.float32)
    w1p[0:CIN] = w1[:, :, 3:6].transpose(1, 2, 0).reshape(CIN, 384)
    w1p[CIN:] = w1[:, :, 0:3].transpose(1, 2, 0).reshape(CIN, 384)
    w1q = np.zeros((2 * CIN, 384), np.float32)
    w1q[CIN:] = w1[:, :, 6:9].transpose(1, 2, 0).reshape(CIN, 384)

    def rnd11(a):
        u = a.view(np.uint32)
        return ((u + np.uint32(1 << 11)) & np.uint32(0xFFFFF000)).view(np.float32)

    for full, ohi, ores in ((w1p, O_W1PH, O_W1PR), (w1q, O_W1QH, O_W1QR)):
        hi = rnd11(full)
        wpk[:, ohi:ohi + 384] = hi
        wpk[:, ores:ores + 384] = full - hi
    wpk[:, O_W2:O_W2 + 1152] = w2.transpose(1, 2, 0).reshape(COUT, 1152)
    wpk[0:CIN, O_WS:O_WS + COUT] = ws.T
    for i, p in enumerate(["bn1_g", "bn1_b", "bn2_g", "bn2_b",
                           "scn_g", "scn_b"]):
        wpk[:, O_PAR + i] = np.asarray(inputs[p], np.float32).ravel()
    return xeo, wpk


_CACHE = {}


def kernel(**inputs):
    xeo, wpk = _prep_inputs(inputs)
    d = float(1.0 / (1.0 + math.exp(-float(np.asarray(inputs["decay"]).ravel()[0]))))

    key = round(d, 12)
    if key not in _CACHE:
        _CACHE[key] = build_nc(d)
    nc = _CACHE[key]

    in_maps = [{"x": xeo[c * BPC:(c + 1) * BPC], "wpk": wpk}
               for c in range(NCORES)]
    res = run_bass_kernel_spmd(nc, in_maps, core_ids=list(range(NCORES)))
    out = np.concatenate([res.results[c]["out"] for c in range(NCORES)], axis=0)
    return out.astype(np.float32)


# revision 26
# speedup vs baseline: 1.0880x; 1.0795x over previous
"""LIAFResBlock forward on 8 Trainium2 NeuronCores (data-parallel over batch).

Self-contained: hardcodes shapes for x [16,64,8,56,56] -> out [16,128,8,28,28].

Math notes (vs the PyTorch/JAX reference):
  - conv biases are no-ops: every conv is followed by training-mode BN, which
    subtracts the per-channel mean, absorbing any per-channel constant.
  - the final mem_update on a binary {0,1} tensor is the identity because
    d = sigmoid(0.5) ~ 0.6225 and d*0.5 < 0.5, so out = lif_act(bn2(cv2)+bn_sc(sc)).
  - the first mem_update runs in "normalized" space: with a1 = g1*rstd1 (>0),
    v = m/a1 satisfies v[t] = d*v[t-1]*[v<=tau] + (cv1[t] + beta1/a1),
    spike[t] = v[t] > tau, tau = 0.5/a1. BN1 folds into a per-channel bias on
    cv1 plus a per-channel threshold.
  - the final compare is sign-safe: out = 1[a2*cv2 > (0.5 - bsc - asc*sc) - b2]
    needs no assumption on a2's sign.
  - BN batch stats are global over B=16: each core computes per-channel
    (sum, sumsq) partials; an AllGather + local sum combines them (cheaper
    than AllReduce on trn2 for tiny payloads).

Layout notes:
  - all weights and BN params are packed on the host into one [128, 2822]
    fp32 array ("wpk", declared float32r) so a single contiguous DMA stages
    them (a DMA-transpose of [O,I,K] weights costs ~100us in 4B descriptors).
  - x is pre-padded and parity-split on the host: per (sample, t) the SBUF
    tile is [128, 29, 58] with partitions 0-63 = even input rows (xe[r] =
    x[2r]) and partitions 64-127 = odd rows shifted (xo[r] = x[2r-1], row 0
    zero). For the stride-2 3x3 conv, out row i needs x rows 2i-1 (xo[i]),
    2i (xe[i]), 2i+1 (xo[i+1]), so taps (kh=0,kh=1) pair into one K=128
    matmul at identical free offsets, and kh=2 rides K=128 matmuls with the
    xe half's weights zeroed. Columns are zero-padded to 58 on the host, so
    no on-chip memsets or duplicated HBM reads are needed.
  - all conv matmuls run as float32r (1 cycle/row for >=256 output rows vs 4
    for fp32). f32r rounds each operand to ~12 mantissa bits, which the LIF
    recurrence amplifies ~27x into spike flips, so conv1 is error-compensated
    in 3 f32r passes: w_hi@x_hi + w_hi@x_res + w_res@x_hi, where x_hi is an
    on-chip DVE f32r copy (bit-identical to the PE's rounding, so x_hi+x_res
    is exact) and w_hi is host-rounded to 11 mantissa bits. The shortcut and
    conv2 stay single-pass f32r: their errors hit only the final threshold
    (no recurrence amplification) and cost ~550 of the ~1858 allowed flips.
"""
import math
import sys

import numpy as np

sys.path.insert(0, "/opt/trn_rl_repo")

import concourse.bass as bass  # noqa: E402
import concourse.bacc as bacc  # noqa: E402
import concourse.tile as tile  # noqa: E402
from concourse import mybir  # noqa: E402
from concourse.bass_utils import run_bass_kernel_spmd  # noqa: E402

dt = mybir.dt
Alu = mybir.AluOpType
Act = mybir.ActivationFunctionType

B, CIN, COUT, T, H, W = 16, 64, 128, 8, 56, 56
HO = WO = 28
NPIX = HO * WO          # 784
CHUNK = NPIX // 2       # 392 (one PSUM bank)
NCORES = 8
BPC = B // NCORES       # 2 samples per core
NT = BPC * T            # 16 (s,t) tiles per core
NLOC = BPC * T * NPIX   # 12544 elements/channel per core
NGLOB = B * T * NPIX    # 100352 elements/channel globally
EPS = 1e-5
XR, XC = 29, 58         # parity-split padded x tile rows/cols
HS = WS = HO + 2        # 30x30 padded spike tile

# packed weight/param column offsets (all in fp32 elements).
# conv1 weights are split on the host into an 11-mantissa-bit "hi" part
# (read exactly by the ~12-bit f32r PE datapath) and the fp32 residual, for
# the 3-pass error-compensated conv1: w_hi*x_hi + w_hi*x_res + w_res*x_hi.
O_W1PH = 0              # [128, 3*128]  conv1 hi taps kh=1 (xe half) / kh=0 (xo)
O_W1QH = 384            # [128, 3*128]  conv1 hi taps kh=2 (xo half; xe half 0)
O_W1PR = 768            # [128, 3*128]  conv1 residual taps (pair 1)
O_W1QR = 1152           # [128, 3*128]  conv1 residual taps (pair 2)
O_W2 = 1536             # [128, 9*128]  conv2 lhsT per tap
O_WS = 2688             # [64, 128]     shortcut lhsT (partitions 0-63)
O_PAR = 2816            # [128, 6]      bn1_g, bn1_b, bn2_g, bn2_b, scn_g, scn_b
WPK = O_PAR + 6


def _ap(base, off, free):
    """Sub-view of a 2D/3D SBUF AP: keep partition dim, custom free dims."""
    return bass.AP(tensor=base.tensor, offset=base.offset + off,
                   ap=[base.ap[0]] + free)


def build_nc(d: float) -> bass.Bass:
    nc = bacc.Bacc("TRN2", target_bir_lowering=False, num_devices=NCORES)
    f32r = dt.float32r

    x_d = nc.dram_tensor("x", [BPC, T, 2 * CIN, XR, XC], dt.float32,
                         kind="ExternalInput")
    wpk_d = nc.dram_tensor("wpk", [2 * CIN, WPK], dt.float32r,
                           kind="ExternalInput")
    out_d = nc.dram_tensor("out", [BPC, COUT, T, HO, WO], dt.float32,
                           kind="ExternalOutput")

    from contextlib import ExitStack
    with tile.TileContext(nc) as tc, ExitStack() as stk:
        big = stk.enter_context(tc.tile_pool(name="big", bufs=1))
        const = stk.enter_context(tc.tile_pool(name="const", bufs=1))
        psum = stk.enter_context(tc.tile_pool(name="psum", bufs=8, space="PSUM"))
        dramp = stk.enter_context(tc.tile_pool(name="dramp", bufs=1, space="DRAM"))

        # ---- first x tile DMA ahead of the weight pack (head latency) ----
        xpool_cm = tc.tile_pool(name="xtp", bufs=2)
        xpool = xpool_cm.__enter__()
        xhp_cm = tc.tile_pool(name="xhp", bufs=3)
        xhp = xhp_cm.__enter__()
        xrp_cm = tc.tile_pool(name="xrp", bufs=3)
        xrp = xrp_cm.__enter__()
        xmp_cm = tc.tile_pool(name="xmp", bufs=3)
        xmp = xmp_cm.__enter__()
        xt00 = xpool.tile([2 * CIN, XR, XC], dt.float32, tag="xt")
        nc.sync.dma_start(out=xt00[:, :, :], in_=x_d[0, 0, :, :, :])

        # ---- weights + params: conv1 part first, rest lands during tile 0
        wpk = const.tile([2 * CIN, WPK], f32r)
        nc.sync.dma_start(out=wpk[:, 0:O_W2], in_=wpk_d[:, 0:O_W2])
        nc.sync.dma_start(out=wpk[:, O_W2:WPK], in_=wpk_d[:, O_W2:WPK])
        wb = wpk[:, 0]
        wbs = wpk[0:CIN, 0]

        def wcol(base, kw):
            return _ap(wb, base + kw * COUT, [[1, COUT]])

        def w2_t(k):
            return _ap(wb, O_W2 + k * COUT, [[1, COUT]])

        ws_t = _ap(wbs, O_WS, [[1, COUT]])
        params = {}
        for i, p in enumerate(["bn1_g", "bn1_b", "bn2_g", "bn2_b",
                               "scn_g", "scn_b"]):
            params[p] = wpk[:, O_PAR + i:O_PAR + i + 1].bitcast(dt.float32)
        eps_t = const.tile([COUT, 1], dt.float32)
        nc.vector.memset(eps_t[:, :], EPS)

        # ---- persistent activation buffers (per-channel partition layout) ----
        cv1f = big.tile([COUT, NLOC], dt.float32)   # conv1 raw, later c' = cv1+beta~
        scf = big.tile([COUT, NLOC], dt.float32)    # shortcut raw, later sc''
        cv2f = cv1f  # conv2 output overwrites c' slices (dead after the v read)
        st1 = const.tile([COUT, 2 * NT, 6], dt.float32)   # bn_stats conv1
        sts = const.tile([COUT, 2 * NT, 6], dt.float32)   # bn_stats shortcut
        st2 = const.tile([COUT, 2 * NT, 6], dt.float32)   # bn_stats conv2

        # persistent spike tiles (rings zeroed once, reused round-robin)
        sps = [big.tile([COUT, HS, WS], f32r, name=f"sp{i}") for i in range(4)]
        for sp in sps:
            for r in (sp[:, 0, :], sp[:, HS - 1, :],
                      _ap(sp[:, 0, 0], 0, [[WS, HS], [WS - 1, 2]])):
                nc.gpsimd.memset(r.bitcast(dt.float32), 0.0)
                nc.gpsimd.tensor_copy(r, r.bitcast(dt.float32))

        # ================= phase A: conv1 + shortcut =================
        # Software-pipelined: the x split for tile i+1 (DMA, f32r hi copy,
        # residual, mixed kh2 tile) is emitted before tile i's matmuls so the
        # in-order DVE/Pool/DMA queues stay ahead of the PE.
        atiles = [(s, t) for s in range(BPC) for t in range(T)]
        prep = {}

        def emit_prep(i):
            s_, t_ = atiles[i]
            if i == 0:
                xt = xt00
            else:
                xt = xpool.tile([2 * CIN, XR, XC], dt.float32, tag="xt")
                nc.sync.dma_start(out=xt[:, :, :], in_=x_d[s_, t_, :, :, :])
            # split x into the f32r-exact hi part (DVE f32r copy == PE
            # rounding) and the fp32 residual (Pool), then 3-pass conv1:
            # w_hi*x_hi + w_hi*x_res + w_res*x_hi
            xh = xhp.tile([2 * CIN, XR, XC], f32r, tag="xh")
            nc.vector.tensor_copy(xh[:, :, :], xt[:, :, :])
            xr = xrp.tile([2 * CIN, XR, XC], f32r, tag="xr")
            nc.vector.tensor_tensor(xr[:, :, :], xt[:, :, :],
                                    xh[:, :, :].bitcast(dt.float32),
                                    Alu.subtract)
            # mixed residual tile: the kh2 taps of passes 2+3 both use only
            # the xo half, so pack [xo_res | xo_hi] and fold both half-empty
            # matmul groups into one full-K group
            xm = xmp.tile([2 * CIN, XR, XC], f32r, tag="xm")
            nc.sync.dma_start(out=xm[0:CIN, :, :], in_=xr[CIN:2 * CIN, :, :])
            nc.gpsimd.tensor_copy(xm[CIN:2 * CIN, :, :], xh[CIN:2 * CIN, :, :])
            prep[i] = (xh, xr, xm)

        emit_prep(0)
        emit_prep(1)
        if True:
            for ia in range(len(atiles)):
                if ia + 2 < len(atiles):
                    emit_prep(ia + 2)
                s, t = atiles[ia]
                it = s * T + t
                xh, xr, xm = prep.pop(ia)
                if True:
                    xhb = xh[:, 0, 0]        # full 128-partition base
                    xrb = xr[:, 0, 0]
                    xmb = xm[:, 0, 0]
                    xe = xh[0:CIN, 0, 0]     # even-rows half (partitions 0-63)
                    for c in range(2):
                        co = c * 14 * XC
                        ps1 = psum.tile([COUT, CHUNK], dt.float32, tag="mm")
                        n = 0
                        # pass 1: w_hi * x_hi (paired taps + kh2)
                        for kw in range(3):
                            rhs = _ap(xhb, kw + co, [[XC, 14], [2, WO]])
                            nc.tensor.matmul(ps1[:, :], wcol(O_W1PH, kw), rhs,
                                             start=(n == 0), stop=False)
                            n += 1
                        for kw in range(3):
                            rhs = _ap(xhb, XC + kw + co, [[XC, 14], [2, WO]])
                            nc.tensor.matmul(ps1[:, :], wcol(O_W1QH, kw), rhs,
                                             start=False, stop=False)
                            n += 1
                        # pass 2/3, paired taps: w_hi * x_res and w_res * x_hi
                        for kw in range(3):
                            rhs = _ap(xrb, kw + co, [[XC, 14], [2, WO]])
                            nc.tensor.matmul(ps1[:, :], wcol(O_W1PH, kw), rhs,
                                             start=False, stop=False)
                            n += 1
                        for kw in range(3):
                            rhs = _ap(xhb, kw + co, [[XC, 14], [2, WO]])
                            nc.tensor.matmul(ps1[:, :], wcol(O_W1PR, kw), rhs,
                                             start=False, stop=False)
                            n += 1
                        # pass 2/3, kh2 taps merged: [w_hi|w_res]*[xo_res|xo_hi]
                        for kw in range(3):
                            rhs = _ap(xmb, XC + kw + co, [[XC, 14], [2, WO]])
                            nc.tensor.matmul(ps1[:, :], wcol(O_W1QR, kw), rhs,
                                             start=False, stop=(n == 15))
                            n += 1
                        off = it * NPIX + c * CHUNK
                        nc.scalar.copy(cv1f[:, off:off + CHUNK], ps1[:, :])
                        nc.vector.bn_stats(out=st1[:, 2 * it + c, :], in_=ps1[:, :])
                        # shortcut 1x1 stride2: x[2i,2j] = xe row i, padded col 2j+1
                        ps2 = psum.tile([COUT, CHUNK], dt.float32, tag="mm")
                        rhs = _ap(xe, 1 + co, [[XC, 14], [2, WO]])
                        nc.tensor.matmul(ps2[:, :], ws_t, rhs,
                                         start=True, stop=True)
                        nc.scalar.copy(scf[:, off:off + CHUNK], ps2[:, :])
                        nc.vector.bn_stats(out=sts[:, 2 * it + c, :], in_=ps2[:, :])

        xmp_cm.__exit__(None, None, None)
        xrp_cm.__exit__(None, None, None)
        xhp_cm.__exit__(None, None, None)
        xpool_cm.__exit__(None, None, None)

        # ---- local stats -> (sum, sumsq) -> AllGather #1 + local sum ----
        # ar1 columns: [sum1, sumsq1, sums, sumsqs]
        mv1 = const.tile([COUT, 2, 2], dt.float32)  # [:, {bn1,scn}, {mean,var}]
        nc.vector.bn_aggr(out=mv1[:, 0, :], in_=st1[:, :, :])
        nc.vector.bn_aggr(out=mv1[:, 1, :], in_=sts[:, :, :])
        ar1 = const.tile([COUT, 2, 2], dt.float32)  # [:, {bn1,scn}, {sum,sumsq}]
        mvb = mv1[:, 0, 0]
        arb = ar1[:, 0, 0]
        mean2 = _ap(mvb, 0, [[2, 2]])
        var2 = _ap(mvb, 1, [[2, 2]])
        sum2 = _ap(arb, 0, [[2, 2]])
        ssq2 = _ap(arb, 1, [[2, 2]])
        nc.vector.tensor_scalar_mul(sum2, mean2, float(NLOC))
        nc.vector.scalar_tensor_tensor(ssq2, mean2, float(NLOC), mean2,
                                       Alu.mult, Alu.mult)
        nc.vector.scalar_tensor_tensor(ssq2, var2, float(NLOC), ssq2,
                                       Alu.mult, Alu.add)

        def gather_sum(ar, width, tag):
            """AllGather [COUT,width] partials from 8 cores, sum locally."""
            cci = dramp.tile([COUT, width], dt.float32, tag=tag + "_i")
            cco = dramp.tile([NCORES, COUT, width], dt.float32,
                             addr_space="Shared", tag=tag + "_o")
            nc.sync.dma_start(out=cci[:, :], in_=ar)
            nc.gpsimd.collective_compute(
                "AllGather", Alu.bypass, replica_groups=[list(range(NCORES))],
                ins=[cci[:, :].opt()], outs=[cco[:, :, :].opt()])
            g = const.tile([COUT, NCORES, width], dt.float32, tag=tag + "_g")
            nc.sync.dma_start(out=g[:, :, :],
                              in_=cco[:, :, :].rearrange("r c k -> c r k"))
            gf = g[:, 0, 0]
            h = NCORES * width
            acc = const.tile([COUT, h // 2], dt.float32, tag=tag + "_a")
            nc.vector.tensor_tensor(
                acc[:, :], _ap(gf, 0, [[1, h // 2]]),
                _ap(gf, h // 2, [[1, h // 2]]), Alu.add)
            while h > 2 * width:
                h //= 2
                af = acc[:, 0]
                nc.vector.tensor_tensor(
                    acc[:, 0:h // 2], _ap(af, 0, [[1, h // 2]]),
                    _ap(af, h // 2, [[1, h // 2]]), Alu.add)
            return acc, g  # acc[:, 0:width] holds the global sums

        gs1, g1raw = gather_sum(ar1[:, :, :], 4, "cc1")

        def mk_bn_consts(sums, g, b, tag, ncol=1):
            """global interleaved (sum, sumsq) -> a = g*rstd, bb = b - a*mean.

            sums is an AP pair (sum_cols, sumsq_cols) of ncol columns each;
            g, b are [COUT, ncol] APs. Returns (a, bb) [COUT, ncol] tiles.
            """
            sum_c, ssq_c = sums
            mean = const.tile([COUT, ncol], dt.float32, tag=tag + "_mean")
            nc.vector.tensor_scalar_mul(mean[:, :], sum_c, 1.0 / NGLOB)
            var = const.tile([COUT, ncol], dt.float32, tag=tag + "_var")
            nc.vector.tensor_scalar_mul(var[:, :], ssq_c, 1.0 / NGLOB)
            m2 = const.tile([COUT, ncol], dt.float32, tag=tag + "_m2")
            nc.vector.tensor_tensor(m2[:, :], mean[:, :], mean[:, :], Alu.mult)
            nc.vector.tensor_tensor(var[:, :], var[:, :], m2[:, :], Alu.subtract)
            a = const.tile([COUT, ncol], dt.float32, tag=tag + "_a")
            nc.scalar.activation(a[:, :], var[:, :], Act.Sqrt,
                                 bias=eps_t[:, 0:1])
            nc.vector.reciprocal(a[:, :], a[:, :])
            nc.vector.tensor_tensor(a[:, :], a[:, :], g, Alu.mult)
            bb = const.tile([COUT, ncol], dt.float32, tag=tag + "_bb")
            nc.vector.tensor_tensor(bb[:, :], a[:, :], mean[:, :], Alu.mult)
            nc.vector.tensor_tensor(bb[:, :], b, bb[:, :], Alu.subtract)
            return a, bb

        # bn1 + scn in one 2-column pass: gs1 cols are [s1, q1, ss, qs];
        # params (bn1_g, scn_g) and (bn1_b, scn_b) are stride-4 pairs in wpk
        g1b = gs1[:, 0]
        a12, b12 = mk_bn_consts(
            (_ap(g1b, 0, [[2, 2]]), _ap(g1b, 1, [[2, 2]])),
            _ap(wb, O_PAR + 0, [[4, 2]]).bitcast(dt.float32),
            _ap(wb, O_PAR + 1, [[4, 2]]).bitcast(dt.float32),
            "bn1scn", ncol=2)
        a1, asc = a12[:, 0:1], a12[:, 1:2]
        b1, bsc = b12[:, 0:1], b12[:, 1:2]

        # tau = 0.5/a1 ; beta~ = b1/a1  (a1 > 0 since gamma=1 at init)
        ra1 = const.tile([COUT, 1], dt.float32)
        nc.vector.reciprocal(ra1[:, :], a1)
        tau = const.tile([COUT, 1], dt.float32)
        nc.vector.tensor_scalar_mul(tau[:, :], ra1[:, :], 0.5)
        btil = const.tile([COUT, 1], dt.float32)
        nc.vector.tensor_tensor(btil[:, :], b1, ra1[:, :], Alu.mult)
        # for the phase-B shortcut fold: sc'' = -asc*sc + (0.5 - bsc)
        nasc = const.tile([COUT, 1], dt.float32)
        nc.vector.tensor_scalar_mul(nasc[:, :], asc, -1.0)
        c1 = const.tile([COUT, 1], dt.float32)
        nc.vector.tensor_scalar(c1[:, :], bsc, -1.0, 0.5, Alu.mult, Alu.add)

        # keep the PE pstate warm through the AR1 window: a few fp32 dummy
        # matmuls gated on the gathered stats (g tile) run right after the
        # collective lands, bridging the gap until the first conv2 is ready.
        for i in range(2):
            pw = psum.tile([COUT, CHUNK], dt.float32, tag="mm")
            nc.tensor.matmul(pw[:, :], wpk[:, 0:COUT].bitcast(dt.float32),
                             _ap(g1raw[:, 0, 0], 0, [[0, 14], [1, 28]]),
                             start=True, stop=True)

        # ================= phase B: LIF recurrence + conv2 =================
        # Software-pipelined with 2-tile lookahead: the recurrence + spike for
        # tile i+2 is emitted before tile i's PSUM copies, so on the in-order
        # Pool/DVE queues spikes never wait behind copies that depend on the
        # PE, and the PE never starves.
        tiles = [(t, s) for t in range(T) for s in range(BPC)]
        v_prev = [None] * BPC
        spof = [None] * len(tiles)

        with tc.tile_pool(name="phu", bufs=3) as pu, \
             tc.tile_pool(name="phv", bufs=5) as pv:

            def emit_front(i):
                """fold + recurrence + spike for tile i."""
                t, s = tiles[i]
                off = (s * T + t) * NPIX
                cslice = cv1f[:, off:off + NPIX]
                nc.scalar.activation(cslice, cslice, Act.Identity,
                                     bias=btil[:, :])
                if t == 0:
                    v = cslice
                else:
                    u = pu.tile([COUT, NPIX], dt.float32, tag="u")
                    nc.vector.scalar_tensor_tensor(
                        u[:, :], v_prev[s], tau[:, :], v_prev[s],
                        Alu.is_le, Alu.mult)
                    vt = pv.tile([COUT, NPIX], dt.float32, tag="v")
                    nc.vector.scalar_tensor_tensor(
                        vt[:, :], u[:, :], float(d), cslice, Alu.mult, Alu.add)
                    v = vt[:, :]
                v_prev[s] = v
                sp = sps[i % 4]
                spof[i] = sp
                spi = _ap(sp[:, 0, 0], WS + 1, [[WS, HO], [1, WO]])
                nc.gpsimd.tensor_scalar(spi, v, tau[:, :], None, Alu.is_gt)

            def emit_back(i):
                """conv2 matmuls + copies + stats + shortcut fold for tile i."""
                t, s = tiles[i]
                it = s * T + t
                off = it * NPIX
                spb = spof[i][:, 0, 0]
                for c in range(2):
                    ps3 = psum.tile([COUT, CHUNK], dt.float32, tag="mm")
                    for k in range(9):
                        kh, kw = divmod(k, 3)
                        rhs = _ap(spb, kh * WS + kw + c * 14 * WS,
                                  [[WS, 14], [1, WO]])
                        nc.tensor.matmul(ps3[:, :], w2_t(k), rhs,
                                         start=(k == 0), stop=(k == 8))
                    o2 = off + c * CHUNK
                    nc.scalar.copy(cv2f[:, o2:o2 + CHUNK], ps3[:, :])
                    nc.vector.bn_stats(out=st2[:, 2 * it + c, :], in_=ps3[:, :])
                # sc'' = -asc*sc + (0.5 - bsc)  (Act slack during phase B)
                nc.scalar.activation(scf[:, off:off + NPIX],
                                     scf[:, off:off + NPIX], Act.Identity,
                                     scale=nasc[:, :], bias=c1[:, :])

            emit_front(0)
            emit_front(1)
            for i in range(len(tiles)):
                if i + 2 < len(tiles):
                    emit_front(i + 2)
                emit_back(i)

        # ---- AllGather #2 (bn2 stats) ----
        mv2 = const.tile([COUT, 2], dt.float32)
        nc.vector.bn_aggr(out=mv2[:, :], in_=st2[:, :, :])
        ar2 = const.tile([COUT, 2], dt.float32)
        nc.vector.tensor_scalar_mul(ar2[:, 0:1], mv2[:, 0:1], float(NLOC))
        nc.vector.scalar_tensor_tensor(ar2[:, 1:2], mv2[:, 0:1], float(NLOC),
                                       mv2[:, 0:1], Alu.mult, Alu.mult)
        nc.vector.scalar_tensor_tensor(ar2[:, 1:2], mv2[:, 1:2], float(NLOC),
                                       ar2[:, 1:2], Alu.mult, Alu.add)
        gs2, _ = gather_sum(ar2[:, :], 2, "cc2")

        a2t, b2t = mk_bn_consts((gs2[:, 0:1], gs2[:, 1:2]),
                                params["bn2_g"], params["bn2_b"], "bn2")
        a2 = a2t[:, 0:1]
        negb2 = const.tile([COUT, 1], dt.float32)
        nc.vector.tensor_scalar_mul(negb2[:, :], b2t[:, 0:1], -1.0)

        # out = 1[a2*cv2 > sc'' - b2]
        with tc.tile_pool(name="outp", bufs=5) as op, \
             tc.tile_pool(name="thp", bufs=5) as tp:
            for s in range(BPC):
                for t in range(T):
                    off = (s * T + t) * NPIX
                    thr = tp.tile([COUT, NPIX], dt.float32, tag="th")
                    if t % 2 == 0:
                        nc.scalar.activation(thr[:, :], scf[:, off:off + NPIX],
                                             Act.Identity, bias=negb2[:, :])
                    else:
                        nc.gpsimd.tensor_scalar(thr[:, :],
                                                scf[:, off:off + NPIX],
                                                negb2[:, :], None, Alu.add)
                    ot = op.tile([COUT, NPIX], dt.float32, tag="ot")
                    nc.vector.scalar_tensor_tensor(
                        ot[:, :], cv2f[:, off:off + NPIX], a2,
                        thr[:, :], Alu.mult, Alu.is_gt)
                    nc.sync.dma_start(
                        out=out_d.ap()[s, :, t, :, :].rearrange("c h w -> c (h w)"),
                        in_=ot[:, :])

    nc.compile()
    return nc


def _prep_inputs(inputs):
    """Host-side restaging: parity-split padded x + packed transposed weights."""
    x = np.ascontiguousarray(inputs["x"], dtype=np.float32)
    xt = x.transpose(0, 2, 1, 3, 4)  # [B, T, C, H, W]
    xeo = np.zeros((B, T, 2 * CIN, XR, XC), dtype=np.float32)
    xeo[:, :, 0:CIN, 0:28, 1:57] = xt[:, :, :, 0::2, :]
    xeo[:, :, CIN:2 * CIN, 1:29, 1:57] = xt[:, :, :, 1::2, :]

    w1 = np.ascontiguousarray(inputs["cv1_w"], np.float32).reshape(COUT, CIN, 9)
    w2 = np.ascontiguousarray(inputs["cv2_w"], np.float32).reshape(COUT, COUT, 9)
    ws = np.ascontiguousarray(inputs["sc_w"], np.float32).reshape(COUT, CIN)
    wpk = np.zeros((2 * CIN, WPK), np.float32)
    # parity-paired conv1 lhsT: kh=1 taps on xe half (partitions 0-63),
    # kh=0 on xo half; kh=2 on xo half with the xe half zero
    w1p = np.zeros((2 * CIN, 384), np.float32)
    w1p[0:CIN] = w1[:, :, 3:6].transpose(1, 2, 0).reshape(CIN, 384)
    w1p[CIN:] = w1[:, :, 0:3].transpose(1, 2, 0).reshape(CIN, 384)
    w1q = np.zeros((2 * CIN, 384), np.float32)
    w1q[CIN:] = w1[:, :, 6:9].transpose(1, 2, 0).reshape(CIN, 384)

    def rnd11(a):
        u = a.view(np.uint32)
        return ((u + np.uint32(1 << 11)) & np.uint32(0xFFFFF000)).view(np.float32)

    hi = rnd11(w1p)
    wpk[:, O_W1PH:O_W1PH + 384] = hi
    wpk[:, O_W1PR:O_W1PR + 384] = w1p - hi
    hiq = rnd11(w1q)
    wpk[:, O_W1QH:O_W1QH + 384] = hiq
    # merged residual block: partitions 0-63 pair xo_res (so they carry the
    # kh2 HI weights), 64-127 pair xo_hi (kh2 residual weights)
    wpk[0:CIN, O_W1QR:O_W1QR + 384] = hiq[CIN:2 * CIN]
    wpk[CIN:2 * CIN, O_W1QR:O_W1QR + 384] = (w1q - hiq)[CIN:2 * CIN]
    wpk[:, O_W2:O_W2 + 1152] = w2.transpose(1, 2, 0).reshape(COUT, 1152)
    wpk[0:CIN, O_WS:O_WS + COUT] = ws.T
    for i, p in enumerate(["bn1_g", "bn1_b", "bn2_g", "bn2_b",
                           "scn_g", "scn_b"]):
        wpk[:, O_PAR + i] = np.asarray(inputs[p], np.float32).ravel()
    return xeo, wpk


_CACHE = {}


def kernel(**inputs):
    xeo, wpk = _prep_inputs(inputs)
    d = float(1.0 / (1.0 + math.exp(-float(np.asarray(inputs["decay"]).ravel()[0]))))

    key = round(d, 12)
    if key not in _CACHE:
        _CACHE[key] = build_nc(d)
    nc = _CACHE[key]

    in_maps = [{"x": xeo[c * BPC:(c + 1) * BPC], "wpk": wpk}
               for c in range(NCORES)]
    res = run_bass_kernel_spmd(nc, in_maps, core_ids=list(range(NCORES)))
    out = np.concatenate([res.results[c]["out"] for c in range(NCORES)], axis=0)
    return out.astype(np.float32)


# revision 27
# speedup vs baseline: 1.0936x; 1.0052x over previous
"""LIAFResBlock forward on 8 Trainium2 NeuronCores (data-parallel over batch).

Self-contained: hardcodes shapes for x [16,64,8,56,56] -> out [16,128,8,28,28].

Math notes (vs the PyTorch/JAX reference):
  - conv biases are no-ops: every conv is followed by training-mode BN, which
    subtracts the per-channel mean, absorbing any per-channel constant.
  - the final mem_update on a binary {0,1} tensor is the identity because
    d = sigmoid(0.5) ~ 0.6225 and d*0.5 < 0.5, so out = lif_act(bn2(cv2)+bn_sc(sc)).
  - the first mem_update runs in "normalized" space: with a1 = g1*rstd1 (>0),
    v = m/a1 satisfies v[t] = d*v[t-1]*[v<=tau] + (cv1[t] + beta1/a1),
    spike[t] = v[t] > tau, tau = 0.5/a1. BN1 folds into a per-channel bias on
    cv1 plus a per-channel threshold.
  - the final compare is sign-safe: out = 1[a2*cv2 > (0.5 - bsc - asc*sc) - b2]
    needs no assumption on a2's sign.
  - BN batch stats are global over B=16: each core computes per-channel
    (sum, sumsq) partials; an AllGather + local sum combines them (cheaper
    than AllReduce on trn2 for tiny payloads).

Layout notes:
  - all weights and BN params are packed on the host into one [128, 2822]
    fp32 array ("wpk", declared float32r) so a single contiguous DMA stages
    them (a DMA-transpose of [O,I,K] weights costs ~100us in 4B descriptors).
  - x is pre-padded and parity-split on the host: per (sample, t) the SBUF
    tile is [128, 29, 58] with partitions 0-63 = even input rows (xe[r] =
    x[2r]) and partitions 64-127 = odd rows shifted (xo[r] = x[2r-1], row 0
    zero). For the stride-2 3x3 conv, out row i needs x rows 2i-1 (xo[i]),
    2i (xe[i]), 2i+1 (xo[i+1]), so taps (kh=0,kh=1) pair into one K=128
    matmul at identical free offsets, and kh=2 rides K=128 matmuls with the
    xe half's weights zeroed. Columns are zero-padded to 58 on the host, so
    no on-chip memsets or duplicated HBM reads are needed.
  - all conv matmuls run as float32r (1 cycle/row for >=256 output rows vs 4
    for fp32). f32r rounds each operand to ~12 mantissa bits, which the LIF
    recurrence amplifies ~27x into spike flips, so conv1 is error-compensated
    in 3 f32r passes: w_hi@x_hi + w_hi@x_res + w_res@x_hi, where x_hi is an
    on-chip DVE f32r copy (bit-identical to the PE's rounding, so x_hi+x_res
    is exact) and w_hi is host-rounded to 11 mantissa bits. The shortcut and
    conv2 stay single-pass f32r: their errors hit only the final threshold
    (no recurrence amplification) and cost ~550 of the ~1858 allowed flips.
"""
import math
import sys

import numpy as np

sys.path.insert(0, "/opt/trn_rl_repo")

import concourse.bass as bass  # noqa: E402
import concourse.bacc as bacc  # noqa: E402
import concourse.tile as tile  # noqa: E402
from concourse import mybir  # noqa: E402
from concourse.bass_utils import run_bass_kernel_spmd  # noqa: E402

dt = mybir.dt
Alu = mybir.AluOpType
Act = mybir.ActivationFunctionType

B, CIN, COUT, T, H, W = 16, 64, 128, 8, 56, 56
HO = WO = 28
NPIX = HO * WO          # 784
CHUNK = NPIX // 2       # 392 (one PSUM bank)
NCORES = 8
BPC = B // NCORES       # 2 samples per core
NT = BPC * T            # 16 (s,t) tiles per core
NLOC = BPC * T * NPIX   # 12544 elements/channel per core
NGLOB = B * T * NPIX    # 100352 elements/channel globally
EPS = 1e-5
XR, XC = 29, 58         # parity-split padded x tile rows/cols
HS = WS = HO + 2        # 30x30 padded spike tile

# packed weight/param column offsets (all in fp32 elements).
# conv1 weights are split on the host into an 11-mantissa-bit "hi" part
# (read exactly by the ~12-bit f32r PE datapath) and the fp32 residual, for
# the 3-pass error-compensated conv1: w_hi*x_hi + w_hi*x_res + w_res*x_hi.
O_W1PH = 0              # [128, 3*128]  conv1 hi taps kh=1 (xe half) / kh=0 (xo)
O_W1QH = 384            # [128, 3*128]  conv1 hi taps kh=2 (xo half; xe half 0)
O_W1PR = 768            # [128, 3*128]  conv1 residual taps (pair 1)
O_W1QR = 1152           # [128, 3*128]  conv1 residual taps (pair 2)
O_W2 = 1536             # [128, 9*128]  conv2 lhsT per tap
O_WS = 2688             # [64, 128]     shortcut lhsT (partitions 0-63)
O_PAR = 2816            # [128, 6]      bn1_g, bn1_b, bn2_g, bn2_b, scn_g, scn_b
WPK = O_PAR + 6


def _ap(base, off, free):
    """Sub-view of a 2D/3D SBUF AP: keep partition dim, custom free dims."""
    return bass.AP(tensor=base.tensor, offset=base.offset + off,
                   ap=[base.ap[0]] + free)


def build_nc(d: float) -> bass.Bass:
    nc = bacc.Bacc("TRN2", target_bir_lowering=False, num_devices=NCORES)
    f32r = dt.float32r

    x_d = nc.dram_tensor("x", [BPC, T, 2 * CIN, XR, XC], dt.float32,
                         kind="ExternalInput")
    wpk_d = nc.dram_tensor("wpk", [2 * CIN, WPK], dt.float32r,
                           kind="ExternalInput")
    out_d = nc.dram_tensor("out", [BPC, COUT, T, HO, WO], dt.float32,
                           kind="ExternalOutput")

    from contextlib import ExitStack
    with tile.TileContext(nc) as tc, ExitStack() as stk:
        big = stk.enter_context(tc.tile_pool(name="big", bufs=1))
        const = stk.enter_context(tc.tile_pool(name="const", bufs=1))
        psum = stk.enter_context(tc.tile_pool(name="psum", bufs=8, space="PSUM"))
        dramp = stk.enter_context(tc.tile_pool(name="dramp", bufs=1, space="DRAM"))

        # ---- first x tile DMA ahead of the weight pack (head latency) ----
        xpool_cm = tc.tile_pool(name="xtp", bufs=2)
        xpool = xpool_cm.__enter__()
        xhp_cm = tc.tile_pool(name="xhp", bufs=3)
        xhp = xhp_cm.__enter__()
        xrp_cm = tc.tile_pool(name="xrp", bufs=3)
        xrp = xrp_cm.__enter__()
        xmp_cm = tc.tile_pool(name="xmp", bufs=3)
        xmp = xmp_cm.__enter__()
        xt00 = xpool.tile([2 * CIN, XR, XC], dt.float32, tag="xt")
        nc.sync.dma_start(out=xt00[:, :, :], in_=x_d[0, 0, :, :, :])

        # ---- weights + params: staged so the first matmul (pass-1 hi
        # weights) waits on the smallest possible transfer
        wpk = const.tile([2 * CIN, WPK], f32r)
        nc.sync.dma_start(out=wpk[:, 0:O_W1PR], in_=wpk_d[:, 0:O_W1PR])
        nc.sync.dma_start(out=wpk[:, O_W1PR:O_W2], in_=wpk_d[:, O_W1PR:O_W2])
        nc.sync.dma_start(out=wpk[:, O_W2:WPK], in_=wpk_d[:, O_W2:WPK])
        wb = wpk[:, 0]
        wbs = wpk[0:CIN, 0]

        def wcol(base, kw):
            return _ap(wb, base + kw * COUT, [[1, COUT]])

        def w2_t(k):
            return _ap(wb, O_W2 + k * COUT, [[1, COUT]])

        ws_t = _ap(wbs, O_WS, [[1, COUT]])
        params = {}
        for i, p in enumerate(["bn1_g", "bn1_b", "bn2_g", "bn2_b",
                               "scn_g", "scn_b"]):
            params[p] = wpk[:, O_PAR + i:O_PAR + i + 1].bitcast(dt.float32)
        eps_t = const.tile([COUT, 1], dt.float32)
        nc.vector.memset(eps_t[:, :], EPS)

        # ---- persistent activation buffers (per-channel partition layout) ----
        cv1f = big.tile([COUT, NLOC], dt.float32)   # conv1 raw, later c' = cv1+beta~
        scf = big.tile([COUT, NLOC], dt.float32)    # shortcut raw, later sc''
        cv2f = cv1f  # conv2 output overwrites c' slices (dead after the v read)
        st1 = const.tile([COUT, 2 * NT, 6], dt.float32)   # bn_stats conv1
        sts = const.tile([COUT, 2 * NT, 6], dt.float32)   # bn_stats shortcut
        st2 = const.tile([COUT, 2 * NT, 6], dt.float32)   # bn_stats conv2

        # persistent spike tiles (rings zeroed once, reused round-robin)
        sps = [big.tile([COUT, HS, WS], f32r, name=f"sp{i}") for i in range(4)]
        for sp in sps:
            for r in (sp[:, 0, :], sp[:, HS - 1, :],
                      _ap(sp[:, 0, 0], 0, [[WS, HS], [WS - 1, 2]])):
                nc.gpsimd.memset(r.bitcast(dt.float32), 0.0)
                nc.gpsimd.tensor_copy(r, r.bitcast(dt.float32))

        # ================= phase A: conv1 + shortcut =================
        # Software-pipelined: the x split for tile i+1 (DMA, f32r hi copy,
        # residual, mixed kh2 tile) is emitted before tile i's matmuls so the
        # in-order DVE/Pool/DMA queues stay ahead of the PE.
        atiles = [(s, t) for s in range(BPC) for t in range(T)]
        prep = {}

        def emit_prep(i):
            s_, t_ = atiles[i]
            if i == 0:
                xt = xt00
            else:
                xt = xpool.tile([2 * CIN, XR, XC], dt.float32, tag="xt")
                nc.sync.dma_start(out=xt[:, :, :], in_=x_d[s_, t_, :, :, :])
            # split x into the f32r-exact hi part (DVE f32r copy == PE
            # rounding) and the fp32 residual (Pool), then 3-pass conv1:
            # w_hi*x_hi + w_hi*x_res + w_res*x_hi
            xh = xhp.tile([2 * CIN, XR, XC], f32r, tag="xh")
            nc.vector.tensor_copy(xh[:, :, :], xt[:, :, :])
            xr = xrp.tile([2 * CIN, XR, XC], f32r, tag="xr")
            nc.vector.tensor_tensor(xr[:, :, :], xt[:, :, :],
                                    xh[:, :, :].bitcast(dt.float32),
                                    Alu.subtract)
            # mixed residual tile: the kh2 taps of passes 2+3 both use only
            # the xo half, so pack [xo_res | xo_hi] and fold both half-empty
            # matmul groups into one full-K group
            xm = xmp.tile([2 * CIN, XR, XC], f32r, tag="xm")
            nc.sync.dma_start(out=xm[0:CIN, :, :], in_=xr[CIN:2 * CIN, :, :])
            nc.gpsimd.tensor_copy(xm[CIN:2 * CIN, :, :], xh[CIN:2 * CIN, :, :])
            prep[i] = (xh, xr, xm)

        emit_prep(0)
        emit_prep(1)
        if True:
            for ia in range(len(atiles)):
                if ia + 2 < len(atiles):
                    emit_prep(ia + 2)
                s, t = atiles[ia]
                it = s * T + t
                xh, xr, xm = prep.pop(ia)
                if True:
                    xhb = xh[:, 0, 0]        # full 128-partition base
                    xrb = xr[:, 0, 0]
                    xmb = xm[:, 0, 0]
                    xe = xh[0:CIN, 0, 0]     # even-rows half (partitions 0-63)
                    for c in range(2):
                        co = c * 14 * XC
                        ps1 = psum.tile([COUT, CHUNK], dt.float32, tag="mm")
                        n = 0
                        # pass 1: w_hi * x_hi (paired taps + kh2)
                        for kw in range(3):
                            rhs = _ap(xhb, kw + co, [[XC, 14], [2, WO]])
                            nc.tensor.matmul(ps1[:, :], wcol(O_W1PH, kw), rhs,
                                             start=(n == 0), stop=False)
                            n += 1
                        for kw in range(3):
                            rhs = _ap(xhb, XC + kw + co, [[XC, 14], [2, WO]])
                            nc.tensor.matmul(ps1[:, :], wcol(O_W1QH, kw), rhs,
                                             start=False, stop=False)
                            n += 1
                        # pass 2/3, paired taps: w_hi * x_res and w_res * x_hi
                        for kw in range(3):
                            rhs = _ap(xrb, kw + co, [[XC, 14], [2, WO]])
                            nc.tensor.matmul(ps1[:, :], wcol(O_W1PH, kw), rhs,
                                             start=False, stop=False)
                            n += 1
                        for kw in range(3):
                            rhs = _ap(xhb, kw + co, [[XC, 14], [2, WO]])
                            nc.tensor.matmul(ps1[:, :], wcol(O_W1PR, kw), rhs,
                                             start=False, stop=False)
                            n += 1
                        # pass 2/3, kh2 taps merged: [w_hi|w_res]*[xo_res|xo_hi]
                        for kw in range(3):
                            rhs = _ap(xmb, XC + kw + co, [[XC, 14], [2, WO]])
                            nc.tensor.matmul(ps1[:, :], wcol(O_W1QR, kw), rhs,
                                             start=False, stop=(n == 15))
                            n += 1
                        off = it * NPIX + c * CHUNK
                        nc.scalar.copy(cv1f[:, off:off + CHUNK], ps1[:, :])
                        nc.vector.bn_stats(out=st1[:, 2 * it + c, :], in_=ps1[:, :])
                        # shortcut 1x1 stride2: x[2i,2j] = xe row i, padded col 2j+1
                        ps2 = psum.tile([COUT, CHUNK], dt.float32, tag="mm")
                        rhs = _ap(xe, 1 + co, [[XC, 14], [2, WO]])
                        nc.tensor.matmul(ps2[:, :], ws_t, rhs,
                                         start=True, stop=True)
                        nc.scalar.copy(scf[:, off:off + CHUNK], ps2[:, :])
                        nc.vector.bn_stats(out=sts[:, 2 * it + c, :], in_=ps2[:, :])

        xmp_cm.__exit__(None, None, None)
        xrp_cm.__exit__(None, None, None)
        xhp_cm.__exit__(None, None, None)
        xpool_cm.__exit__(None, None, None)

        # ---- local stats -> (sum, sumsq) -> AllGather #1 + local sum ----
        # ar1 columns: [sum1, sumsq1, sums, sumsqs]
        mv1 = const.tile([COUT, 2, 2], dt.float32)  # [:, {bn1,scn}, {mean,var}]
        nc.vector.bn_aggr(out=mv1[:, 0, :], in_=st1[:, :, :])
        nc.vector.bn_aggr(out=mv1[:, 1, :], in_=sts[:, :, :])
        ar1 = const.tile([COUT, 2, 2], dt.float32)  # [:, {bn1,scn}, {sum,sumsq}]
        mvb = mv1[:, 0, 0]
        arb = ar1[:, 0, 0]
        mean2 = _ap(mvb, 0, [[2, 2]])
        var2 = _ap(mvb, 1, [[2, 2]])
        sum2 = _ap(arb, 0, [[2, 2]])
        ssq2 = _ap(arb, 1, [[2, 2]])
        nc.vector.tensor_scalar_mul(sum2, mean2, float(NLOC))
        nc.vector.scalar_tensor_tensor(ssq2, mean2, float(NLOC), mean2,
                                       Alu.mult, Alu.mult)
        nc.vector.scalar_tensor_tensor(ssq2, var2, float(NLOC), ssq2,
                                       Alu.mult, Alu.add)

        def gather_sum(ar, width, tag):
            """AllGather [COUT,width] partials from 8 cores, sum locally."""
            cci = dramp.tile([COUT, width], dt.float32, tag=tag + "_i")
            cco = dramp.tile([NCORES, COUT, width], dt.float32,
                             addr_space="Shared", tag=tag + "_o")
            nc.sync.dma_start(out=cci[:, :], in_=ar)
            nc.gpsimd.collective_compute(
                "AllGather", Alu.bypass, replica_groups=[list(range(NCORES))],
                ins=[cci[:, :].opt()], outs=[cco[:, :, :].opt()])
            g = const.tile([COUT, NCORES, width], dt.float32, tag=tag + "_g")
            nc.sync.dma_start(out=g[:, :, :],
                              in_=cco[:, :, :].rearrange("r c k -> c r k"))
            gf = g[:, 0, 0]
            h = NCORES * width
            acc = const.tile([COUT, h // 2], dt.float32, tag=tag + "_a")
            nc.vector.tensor_tensor(
                acc[:, :], _ap(gf, 0, [[1, h // 2]]),
                _ap(gf, h // 2, [[1, h // 2]]), Alu.add)
            while h > 2 * width:
                h //= 2
                af = acc[:, 0]
                nc.vector.tensor_tensor(
                    acc[:, 0:h // 2], _ap(af, 0, [[1, h // 2]]),
                    _ap(af, h // 2, [[1, h // 2]]), Alu.add)
            return acc, g  # acc[:, 0:width] holds the global sums

        gs1, g1raw = gather_sum(ar1[:, :, :], 4, "cc1")

        def mk_bn_consts(sums, g, b, tag, ncol=1):
            """global interleaved (sum, sumsq) -> a = g*rstd, bb = b - a*mean.

            sums is an AP pair (sum_cols, sumsq_cols) of ncol columns each;
            g, b are [COUT, ncol] APs. Returns (a, bb) [COUT, ncol] tiles.
            """
            sum_c, ssq_c = sums
            mean = const.tile([COUT, ncol], dt.float32, tag=tag + "_mean")
            nc.vector.tensor_scalar_mul(mean[:, :], sum_c, 1.0 / NGLOB)
            var = const.tile([COUT, ncol], dt.float32, tag=tag + "_var")
            nc.vector.tensor_scalar_mul(var[:, :], ssq_c, 1.0 / NGLOB)
            m2 = const.tile([COUT, ncol], dt.float32, tag=tag + "_m2")
            nc.vector.tensor_tensor(m2[:, :], mean[:, :], mean[:, :], Alu.mult)
            nc.vector.tensor_tensor(var[:, :], var[:, :], m2[:, :], Alu.subtract)
            a = const.tile([COUT, ncol], dt.float32, tag=tag + "_a")
            nc.scalar.activation(a[:, :], var[:, :], Act.Sqrt,
                                 bias=eps_t[:, 0:1])
            nc.vector.reciprocal(a[:, :], a[:, :])
            nc.vector.tensor_tensor(a[:, :], a[:, :], g, Alu.mult)
            bb = const.tile([COUT, ncol], dt.float32, tag=tag + "_bb")
            nc.vector.tensor_tensor(bb[:, :], a[:, :], mean[:, :], Alu.mult)
            nc.vector.tensor_tensor(bb[:, :], b, bb[:, :], Alu.subtract)
            return a, bb

        # bn1 + scn in one 2-column pass: gs1 cols are [s1, q1, ss, qs];
        # params (bn1_g, scn_g) and (bn1_b, scn_b) are stride-4 pairs in wpk
        g1b = gs1[:, 0]
        a12, b12 = mk_bn_consts(
            (_ap(g1b, 0, [[2, 2]]), _ap(g1b, 1, [[2, 2]])),
            _ap(wb, O_PAR + 0, [[4, 2]]).bitcast(dt.float32),
            _ap(wb, O_PAR + 1, [[4, 2]]).bitcast(dt.float32),
            "bn1scn", ncol=2)
        a1, asc = a12[:, 0:1], a12[:, 1:2]
        b1, bsc = b12[:, 0:1], b12[:, 1:2]

        # tau = 0.5/a1 ; beta~ = b1/a1  (a1 > 0 since gamma=1 at init)
        ra1 = const.tile([COUT, 1], dt.float32)
        nc.vector.reciprocal(ra1[:, :], a1)
        tau = const.tile([COUT, 1], dt.float32)
        nc.vector.tensor_scalar_mul(tau[:, :], ra1[:, :], 0.5)
        btil = const.tile([COUT, 1], dt.float32)
        nc.vector.tensor_tensor(btil[:, :], b1, ra1[:, :], Alu.mult)
        # for the phase-B shortcut fold: sc'' = -asc*sc + (0.5 - bsc)
        nasc = const.tile([COUT, 1], dt.float32)
        nc.vector.tensor_scalar_mul(nasc[:, :], asc, -1.0)
        c1 = const.tile([COUT, 1], dt.float32)
        nc.vector.tensor_scalar(c1[:, :], bsc, -1.0, 0.5, Alu.mult, Alu.add)

        # keep the PE pstate warm through the AR1 window: a few fp32 dummy
        # matmuls gated on the gathered stats (g tile) run right after the
        # collective lands, bridging the gap until the first conv2 is ready.
        for i in range(2):
            pw = psum.tile([COUT, CHUNK], dt.float32, tag="mm")
            nc.tensor.matmul(pw[:, :], wpk[:, 0:COUT].bitcast(dt.float32),
                             _ap(g1raw[:, 0, 0], 0, [[0, 14], [1, 28]]),
                             start=True, stop=True)

        # ================= phase B: LIF recurrence + conv2 =================
        # Software-pipelined with 2-tile lookahead: the recurrence + spike for
        # tile i+2 is emitted before tile i's PSUM copies, so on the in-order
        # Pool/DVE queues spikes never wait behind copies that depend on the
        # PE, and the PE never starves.
        tiles = [(t, s) for t in range(T) for s in range(BPC)]
        v_prev = [None] * BPC
        spof = [None] * len(tiles)

        with tc.tile_pool(name="phu", bufs=3) as pu, \
             tc.tile_pool(name="phv", bufs=5) as pv:

            def emit_front(i):
                """fold + recurrence + spike for tile i."""
                t, s = tiles[i]
                off = (s * T + t) * NPIX
                cslice = cv1f[:, off:off + NPIX]
                nc.scalar.activation(cslice, cslice, Act.Identity,
                                     bias=btil[:, :])
                if t == 0:
                    v = cslice
                else:
                    u = pu.tile([COUT, NPIX], dt.float32, tag="u")
                    nc.vector.scalar_tensor_tensor(
                        u[:, :], v_prev[s], tau[:, :], v_prev[s],
                        Alu.is_le, Alu.mult)
                    vt = pv.tile([COUT, NPIX], dt.float32, tag="v")
                    nc.vector.scalar_tensor_tensor(
                        vt[:, :], u[:, :], float(d), cslice, Alu.mult, Alu.add)
                    v = vt[:, :]
                v_prev[s] = v
                sp = sps[i % 4]
                spof[i] = sp
                spi = _ap(sp[:, 0, 0], WS + 1, [[WS, HO], [1, WO]])
                nc.gpsimd.tensor_scalar(spi, v, tau[:, :], None, Alu.is_gt)

            def emit_back(i):
                """conv2 matmuls + copies + stats + shortcut fold for tile i."""
                t, s = tiles[i]
                it = s * T + t
                off = it * NPIX
                spb = spof[i][:, 0, 0]
                for c in range(2):
                    ps3 = psum.tile([COUT, CHUNK], dt.float32, tag="mm")
                    for k in range(9):
                        kh, kw = divmod(k, 3)
                        rhs = _ap(spb, kh * WS + kw + c * 14 * WS,
                                  [[WS, 14], [1, WO]])
                        nc.tensor.matmul(ps3[:, :], w2_t(k), rhs,
                                         start=(k == 0), stop=(k == 8))
                    o2 = off + c * CHUNK
                    nc.scalar.copy(cv2f[:, o2:o2 + CHUNK], ps3[:, :])
                    nc.vector.bn_stats(out=st2[:, 2 * it + c, :], in_=ps3[:, :])
                # sc'' = -asc*sc + (0.5 - bsc)  (Act slack during phase B)
                nc.scalar.activation(scf[:, off:off + NPIX],
                                     scf[:, off:off + NPIX], Act.Identity,
                                     scale=nasc[:, :], bias=c1[:, :])

            emit_front(0)
            emit_front(1)
            for i in range(len(tiles)):
                if i + 2 < len(tiles):
                    emit_front(i + 2)
                emit_back(i)

        # ---- AllGather #2 (bn2 stats) ----
        mv2 = const.tile([COUT, 2], dt.float32)
        nc.vector.bn_aggr(out=mv2[:, :], in_=st2[:, :, :])
        ar2 = const.tile([COUT, 2], dt.float32)
        nc.vector.tensor_scalar_mul(ar2[:, 0:1], mv2[:, 0:1], float(NLOC))
        nc.vector.scalar_tensor_tensor(ar2[:, 1:2], mv2[:, 0:1], float(NLOC),
                                       mv2[:, 0:1], Alu.mult, Alu.mult)
        nc.vector.scalar_tensor_tensor(ar2[:, 1:2], mv2[:, 1:2], float(NLOC),
                                       ar2[:, 1:2], Alu.mult, Alu.add)
        gs2, _ = gather_sum(ar2[:, :], 2, "cc2")

        a2t, b2t = mk_bn_consts((gs2[:, 0:1], gs2[:, 1:2]),
                                params["bn2_g"], params["bn2_b"], "bn2")
        a2 = a2t[:, 0:1]
        negb2 = const.tile([COUT, 1], dt.float32)
        nc.vector.tensor_scalar_mul(negb2[:, :], b2t[:, 0:1], -1.0)

        # out = 1[a2*cv2 > sc'' - b2]
        with tc.tile_pool(name="outp", bufs=5) as op, \
             tc.tile_pool(name="thp", bufs=5) as tp:
            for s in range(BPC):
                for t in range(T):
                    off = (s * T + t) * NPIX
                    thr = tp.tile([COUT, NPIX], dt.float32, tag="th")
                    if t % 2 == 0:
                        nc.scalar.activation(thr[:, :], scf[:, off:off + NPIX],
                                             Act.Identity, bias=negb2[:, :])
                    else:
                        nc.gpsimd.tensor_scalar(thr[:, :],
                                                scf[:, off:off + NPIX],
                                                negb2[:, :], None, Alu.add)
                    ot = op.tile([COUT, NPIX], dt.float32, tag="ot")
                    nc.vector.scalar_tensor_tensor(
                        ot[:, :], cv2f[:, off:off + NPIX], a2,
                        thr[:, :], Alu.mult, Alu.is_gt)
                    nc.sync.dma_start(
                        out=out_d.ap()[s, :, t, :, :].rearrange("c h w -> c (h w)"),
                        in_=ot[:, :])

    nc.compile()
    return nc


def _prep_inputs(inputs):
    """Host-side restaging: parity-split padded x + packed transposed weights."""
    x = np.ascontiguousarray(inputs["x"], dtype=np.float32)
    xt = x.transpose(0, 2, 1, 3, 4)  # [B, T, C, H, W]
    xeo = np.zeros((B, T, 2 * CIN, XR, XC), dtype=np.float32)
    xeo[:, :, 0:CIN, 0:28, 1:57] = xt[:, :, :, 0::2, :]
    xeo[:, :, CIN:2 * CIN, 1:29, 1:57] = xt[:, :, :, 1::2, :]

    w1 = np.ascontiguousarray(inputs["cv1_w"], np.float32).reshape(COUT, CIN, 9)
    w2 = np.ascontiguousarray(inputs["cv2_w"], np.float32).reshape(COUT, COUT, 9)
    ws = np.ascontiguousarray(inputs["sc_w"], np.float32).reshape(COUT, CIN)
    wpk = np.zeros((2 * CIN, WPK), np.float32)
    # parity-paired conv1 lhsT: kh=1 taps on xe half (partitions 0-63),
    # kh=0 on xo half; kh=2 on xo half with the xe half zero
    w1p = np.zeros((2 * CIN, 384), np.float32)
    w1p[0:CIN] = w1[:, :, 3:6].transpose(1, 2, 0).reshape(CIN, 384)
    w1p[CIN:] = w1[:, :, 0:3].transpose(1, 2, 0).reshape(CIN, 384)
    w1q = np.zeros((2 * CIN, 384), np.float32)
    w1q[CIN:] = w1[:, :, 6:9].transpose(1, 2, 0).reshape(CIN, 384)

    def rnd11(a):
        u = a.view(np.uint32)
        return ((u + np.uint32(1 << 11)) & np.uint32(0xFFFFF000)).view(np.float32)

    hi = rnd11(w1p)
    wpk[:, O_W1PH:O_W1PH + 384] = hi
    wpk[:, O_W1PR:O_W1PR + 384] = w1p - hi
    hiq = rnd11(w1q)
    wpk[:, O_W1QH:O_W1QH + 384] = hiq
    # merged residual block: partitions 0-63 pair xo_res (so they carry the
    # kh2 HI weights), 64-127 pair xo_hi (kh2 residual weights)
    wpk[0:CIN, O_W1QR:O_W1QR + 384] = hiq[CIN:2 * CIN]
    wpk[CIN:2 * CIN, O_W1QR:O_W1QR + 384] = (w1q - hiq)[CIN:2 * CIN]
    wpk[:, O_W2:O_W2 + 1152] = w2.transpose(1, 2, 0).reshape(COUT, 1152)
    wpk[0:CIN, O_WS:O_WS + COUT] = ws.T
    for i, p in enumerate(["bn1_g", "bn1_b", "bn2_g", "bn2_b",
                           "scn_g", "scn_b"]):
        wpk[:, O_PAR + i] = np.asarray(inputs[p], np.float32).ravel()
    return xeo, wpk


_CACHE = {}


def kernel(**inputs):
    xeo, wpk = _prep_inputs(inputs)
    d = float(1.0 / (1.0 + math.exp(-float(np.asarray(inputs["decay"]).ravel()[0]))))

    key = round(d, 12)
    if key not in _CACHE:
        _CACHE[key] = build_nc(d)
    nc = _CACHE[key]

    in_maps = [{"x": xeo[c * BPC:(c + 1) * BPC], "wpk": wpk}
               for c in range(NCORES)]
    res = run_bass_kernel_spmd(nc, in_maps, core_ids=list(range(NCORES)))
    out = np.concatenate([res.results[c]["out"] for c in range(NCORES)], axis=0)
    return out.astype(np.float32)
